# revision 18
# baseline (speedup 1.0000x reference)
"""MRI data-consistency CG solver on 8 Trainium2 NeuronCores.

Sharding: pure data-parallel, 1 batch sample per core. The CG alpha/beta
scalars are computed per-sample (deviation from the reference's global
batch sums is ~3e-4 relative, far below tolerance), so cores run fully
independently - no collectives.

Per coil, the centered 2D FFT / IFFT are chained PE matmuls with the
centered DFT matrix Fc (Fc = S F S is symmetric, so
  stage1 = X^T Fc   (data as lhsT; output transposed)
  stage2 = stage1^T Fc = Fc X Fc  (natural orientation again)
-> no explicit transposes anywhere).

v2 data-path: the matmul STATIONARY side (the data: cp/s1/km/s3) is
bf16 - LDWEIGHTS then streams 2-byte rows and fully hides under the
320-col f32r moving operand (G matrices stay f32r => fp32-grade DFT
coefficients; fp32 PSUM accumulate). All csm-coupled elementwise ops
run in bf16 on DVE (2x_1p mode) with the fp32 CG state (r/p/q/b) kept
in fp32. Measured-model final rel err ~5e-3 vs 2e-2 gate.

All 32 csm half-tiles are SBUF-resident in bf16 (no per-iteration
re-streaming). The CG scalars use fused tensor_tensor_reduce (pq) and
ACT Square+accum (qq/rr). The last CG iteration only needs
alpha = rr / p^H A p, and p^H A p = sum_c ||M F C_c p||^2 + mu ||p||^2,
so stages 3/4 + the coil combine are skipped there; the masked-kspace
norms accumulate via chained TTRs.

Field layout: each 320x320 field lives in one SBUF tile [128, 3*320]
("folded"): block b (cols [b*320,(b+1)*320)) holds rows [b*128, ...) of
the matrix. Block 2 only uses partitions 0..63 in the standard layout
(junk kept at 0); the matmul-chain intermediates instead use a "packed"
layout where block 2 of the REAL tile holds [re rows 256:320 (p0:64);
im rows 256:320 (p64:128)], which lets the two 64-row contraction tails
merge into one full 128-row matmul (k2-packing).

CG scalar algebra: alpha is real (p^H A p real) and r^H q == p^H q by
A-conjugacy, so per iteration only two sums are needed:
  pq = sum(q_r p_r + q_i p_i),  qq = sum(|q|^2)
  alpha = rr/pq;  rr_new = alpha^2 qq - rr;  beta = rr_new/rr
"""

import numpy as np

CG_ITER = 10

_nc_cache = {}
LAST_RESULT = None


def _blocks(n):
    out = []
    r0 = 0
    while r0 < n:
        sz = min(128, n - r0)
        out.append((r0, sz))
        r0 += sz
    return out


def _centered_dft(n):
    # Columns of Fc = centered orthonormal DFT applied to unit vectors:
    # y = fftshift(fft(ifftshift(x))) = Fc @ x. Fc is symmetric for even n.
    eye = np.eye(n)
    Fc = np.fft.fftshift(
        np.fft.fft(np.fft.ifftshift(eye, axes=0), axis=0, norm="ortho"), axes=0
    )
    return Fc


def _build(Hc, Wc, Cc, iters, n_cores, reps=1, dma_pack=True):
    import concourse.bacc as bacc
    import concourse.mybir as mybir
    import concourse.tile as tile

    f32 = mybir.dt.float32
    f32r = mybir.dt.float32r
    bf16 = mybir.dt.float16  # "bf16" name kept; fp16 = same speed, 4 more mantissa bits
    OP = mybir.AluOpType
    ACTF = mybir.ActivationFunctionType

    nc = bacc.Bacc(trn_type="TRN2", num_devices=n_cores)

    BL = _blocks(Hc)
    NB = len(BL)
    FW = NB * Wc
    # size of the partial tail block (0 if H divides evenly)
    RTL = BL[-1][1] if BL[-1][1] < 128 else 0
    packing = RTL > 0 and 2 * RTL <= 128
    import os as _os
    if _os.environ.get("NO_PACK"):
        packing = False

    us = nc.dram_tensor("us_image", [2, Hc, Wc], f32, kind="ExternalInput")
    rec = nc.dram_tensor("reconstruction", [2, Hc, Wc], f32, kind="ExternalInput")
    mask_d = nc.dram_tensor("mask", [Hc, Wc], bf16, kind="ExternalInput")
    csm_r_d = nc.dram_tensor("csm_r", [Cc, Hc, Wc], bf16, kind="ExternalInput")
    csm_i_d = nc.dram_tensor("csm_i", [Cc, Hc, Wc], bf16, kind="ExternalInput")
    mu_d = nc.dram_tensor("mu", [1], f32, kind="ExternalInput")
    fr_d = nc.dram_tensor("f_r", [Hc, Hc], bf16, kind="ExternalInput")
    fi_d = nc.dram_tensor("f_i", [Hc, Hc], bf16, kind="ExternalInput")
    fni_d = nc.dram_tensor("f_ni", [Hc, Hc], bf16, kind="ExternalInput")
    if packing:
        # packed k2 rhs tiles: [Ga[tail rows] on p0:R ; Gb[tail] on pR:2R]
        # slots: 0=[fr;fni] 1=[fi;fr] 2=[fr;fi] 3=[fni;fr]
        fpk_d = nc.dram_tensor("f_pk", [2 * RTL, 4 * Wc], bf16, kind="ExternalInput")
        mpk_d = nc.dram_tensor("mask_pk", [2 * RTL, Wc], bf16, kind="ExternalInput")
    out_d = nc.dram_tensor("out", [2, Hc, Wc], f32, kind="ExternalOutput")

    with tile.TileContext(nc) as tc:
        with (
            tc.tile_pool(name="consts", bufs=1) as consts,
            tc.tile_pool(name="state", bufs=1) as state,
            tc.tile_pool(name="work", bufs=1) as work,
            tc.tile_pool(name="small", bufs=1) as small,
            tc.tile_pool(name="psum", bufs=8, space="PSUM") as psp,
        ):
            zero_f32 = []   # [128, FW] f32 tiles to memset (junk must be 0)
            zero_bf16 = []  # bf16 tiles whose junk feeds reductions

            def T(pool, name, shape, dtype=f32, zero=None):
                tl = pool.tile(shape, dtype, tag=name)
                if zero is None:
                    zero = list(shape) == [128, FW] and dtype == f32
                if zero:
                    (zero_f32 if dtype == f32 else zero_bf16).append(tl)
                return tl

            fr = T(consts, "fr", [128, FW], bf16)
            fi = T(consts, "fi", [128, FW], bf16)
            fni = T(consts, "fni", [128, FW], bf16)
            maskf = T(consts, "maskf", [128, FW], bf16)
            if packing:
                fpk = T(consts, "fpk", [2 * RTL, 4 * Wc], bf16)
                mpk = T(consts, "mpk", [2 * RTL, Wc], bf16)
            ones_col = T(consts, "ones_col", [128, 1])
            ones_row = T(consts, "ones_row", [1, 128])
            mu_b = T(consts, "mu_b", [128, 1])
            mu_sb = T(consts, "mu_sb", [1, 1])

            pp = [
                [T(state, "pA_r", [128, FW]), T(state, "pA_i", [128, FW])],
                [T(state, "pB_r", [128, FW]), T(state, "pB_i", [128, FW])],
            ]
            r_r = T(state, "r_r", [128, FW])
            r_i = T(state, "r_i", [128, FW])
            b_r = T(state, "b_r", [128, FW])
            b_i = T(state, "b_i", [128, FW])
            q_r = T(state, "q_r", [128, FW])
            q_i = T(state, "q_i", [128, FW])
            # bf16 shadow of p for the csm products
            pb_r = T(state, "pb_r", [128, FW], bf16, zero=True)
            pb_i = T(state, "pb_i", [128, FW], bf16, zero=True)

            cp = [[T(work, f"cp_{x}{j}", [128, FW], bf16) for x in "ri"] for j in (0, 1)]
            s1 = [[T(work, f"s1_{x}{j}", [128, FW], bf16) for x in "ri"] for j in (0, 1)]
            # km feeds full-tile TTR reductions on the last iteration
            km = [[T(work, f"km_{x}{j}", [128, FW], bf16, zero=True) for x in "ri"]
                  for j in (0, 1)]
            s3 = [[T(work, f"s3_{x}{j}", [128, FW], bf16) for x in "ri"] for j in (0, 1)]
            # zz feeds the fp32 q accumulation: junk must be 0
            zz = [[T(work, f"zz_{x}{j}", [128, FW], bf16, zero=True) for x in "ri"]
                  for j in (0, 1)]
            # all csm tiles SBUF-resident in bf16
            cs = [
                [T(work, f"cs_{x}{j}", [128, FW], bf16, zero=True) for x in "ri"]
                for j in range(Cc)
            ]
            pj1 = T(work, "pj1", [128, FW], bf16)
            pj2 = T(work, "pj2", [128, FW], bf16)
            pj3 = T(work, "pj3", [128, FW], bf16)
            pj4 = T(work, "pj4", [128, FW], bf16)
            tA = T(work, "tA", [128, FW], bf16)
            tB = T(work, "tB", [128, FW], bf16)
            tC = T(work, "tC", [128, FW], bf16)
            tD = T(work, "tD", [128, FW], bf16)
            tE = T(work, "tE", [128, FW], bf16)
            tF = T(work, "tF", [128, FW], bf16)
            dump = T(work, "dump", [128, FW], bf16)

            # cols: 0/1 pq parts, 2/3 qq parts, 4/5 rr parts,
            # 6..6+2C last-iter |masked ksp|^2 per coil, then 2 p-norm cols
            NPART = 8 + 2 * Cc
            partials = T(small, "partials", [128, NPART])
            redsums = T(small, "redsums", [1, NPART])
            scl = T(small, "scl", [1, 8])
            alphas = T(small, "alphas", [1, 4])
            bc = T(small, "bc", [128, 4])
            rr_t = T(small, "rr", [1, 1])
            rrn_t = T(small, "rrn", [1, 1])

            v = nc.vector
            g = nc.gpsimd
            a = nc.scalar
            STT_V = v.scalar_tensor_tensor
            TT = v.tensor_tensor

            # ---- init: zero everything (keeps junk regions at 0)
            for tl in zero_f32 + zero_bf16:
                v.memset(tl, 0.0)
            v.memset(partials, 0.0)
            v.memset(ones_col, 1.0)
            v.memset(ones_row, 1.0)

            def load_folded(dst, src2d):
                nbf = Hc // 128
                full = nbf * 128
                if nbf:
                    nc.sync.dma_start(
                        out=dst[:, 0 : nbf * Wc].rearrange("p (b w) -> p b w", b=nbf),
                        in_=src2d[0:full, :].rearrange("(b p) w -> p b w", p=128),
                    )
                if full < Hc:
                    rem = Hc - full
                    nc.sync.dma_start(
                        out=dst[:rem, nbf * Wc : (nbf + 1) * Wc],
                        in_=src2d[full:Hc, :],
                    )

            def store_folded(src, dst2d):
                nbf = Hc // 128
                full = nbf * 128
                if nbf:
                    nc.sync.dma_start(
                        out=dst2d[0:full, :].rearrange("(b p) w -> p b w", p=128),
                        in_=src[:, 0 : nbf * Wc].rearrange("p (b w) -> p b w", b=nbf),
                    )
                if full < Hc:
                    rem = Hc - full
                    nc.sync.dma_start(
                        out=dst2d[full:Hc, :],
                        in_=src[:rem, nbf * Wc : (nbf + 1) * Wc],
                    )

            # order: init-chain dependencies first (mu, us/rec for r, csm0/1
            # for the first projections), then the DFT matrices needed by
            # stage 1, then the rest
            nc.sync.dma_start(out=mu_sb[:1, :1], in_=mu_d[None, :])
            # borrow pp[1] and b for the r-init staging (all junk pre-zeroed)
            load_folded(pp[1][0], us[0])
            load_folded(pp[1][1], us[1])
            load_folded(b_r, rec[0])
            load_folded(b_i, rec[1])

            psb = psp.tile([128, 16], f32, tag="mm")
            nc.tensor.matmul(
                psb[:, :1], lhsT=ones_row[:1, :128], rhs=mu_sb[:1, :1],
                start=True, stop=True,
            )
            a.copy(out=mu_b[:, :1], in_=psb[:, :1])

            def load_csm(ci_):
                load_folded(cs[ci_][0], csm_r_d[ci_])
                load_folded(cs[ci_][1], csm_i_d[ci_])

            for j in range(min(Cc, 2)):
                load_csm(j)
            load_folded(fr, fr_d[:])
            load_folded(fi, fi_d[:])
            load_folded(fni, fni_d[:])
            if packing:
                nc.sync.dma_start(out=fpk, in_=fpk_d[:])
                nc.sync.dma_start(out=mpk, in_=mpk_d[:])
            load_folded(maskf, mask_d[:])
            for j in range(2, Cc):
                load_csm(j)

            # G-sets: (gr, gi, gni, pk_pr, pk_pi)
            if packing:
                gF = (fr, fi, fni, fpk[:, 0:Wc], fpk[:, Wc : 2 * Wc])
                gB = (fr, fni, fi, fpk[:, 2 * Wc : 3 * Wc], fpk[:, 3 * Wc : 4 * Wc])
            else:
                gF = (fr, fi, fni, None, None)
                gB = (fr, fni, fi, None, None)

            def mm_group2_head(out_a, wa, ra, out_b, wb, rb, has_tail):
                """Interleaved head matmuls of two accumulation groups.

                If has_tail, groups are left open for a deferred k2 pair.
                """
                n = len(wa)
                assert len(wb) == n
                for j in range(n):
                    stop = (not has_tail) and j == n - 1
                    nc.tensor.matmul(
                        out_a, lhsT=wa[j], rhs=ra[j], start=(j == 0), stop=stop,
                    )
                    nc.tensor.matmul(
                        out_b, lhsT=wb[j], rhs=rb[j], start=(j == 0), stop=stop,
                    )

            def mm_stage(xr, xi, gset, packed_in, packed_out, consume):
                """out = (xr + i xi)^T @ (gr + i gi); gni = -gi precomputed.

                packed_in: xr block NB-1 holds [re_tail; im_tail] (k2-packing)
                packed_out: m-tail block's imag half staged for partition shift
                """
                gr, gi, gni, pk_pr, pk_pi = gset

                def wslices(tl, m0, msz):
                    # weight slices per k-block: list of (ap, rhs) pairs
                    return [
                        tl[:ksz, k * Wc + m0 : k * Wc + m0 + msz]
                        for k, (k0, ksz) in enumerate(BL)
                    ]

                pending = []

                def flush_one():
                    # emit the deferred k2 pair of the oldest open m-block,
                    # close its groups, and evacuate
                    m, msz, pr, pi, tail = pending.pop(0)
                    if tail is not None:
                        (wk2, rk2_a, rk2_b) = tail
                        out_b = pi[:msz, :] if pi is not None else pr[64 : 64 + msz, :]
                        nc.tensor.matmul(
                            pr[:msz, :], lhsT=wk2, rhs=rk2_a,
                            start=False, stop=True,
                        )
                        nc.tensor.matmul(
                            out_b, lhsT=wk2, rhs=rk2_b,
                            start=False, stop=True,
                        )
                    consume(m, msz, pr, pi)

                for m in range(NB):
                    m0, msz = BL[m]
                    wr = wslices(xr, m0, msz)
                    wi = wslices(xi, m0, msz)
                    tail = None
                    if packed_in and packing:
                        # last k-block: single 128-row packed MM (deferred -
                        # its input is written by the previous stage's
                        # partition-shift, so give it pipeline slack)
                        w_pr = wr[:-1] + wi[:-1]
                        r_pr = [
                            gr[:128, k * Wc : (k + 1) * Wc] for k in range(NB - 1)
                        ] + [
                            gni[:128, k * Wc : (k + 1) * Wc] for k in range(NB - 1)
                        ]
                        w_pi = wr[:-1] + wi[:-1]
                        r_pi = [
                            gi[:128, k * Wc : (k + 1) * Wc] for k in range(NB - 1)
                        ] + [
                            gr[:128, k * Wc : (k + 1) * Wc] for k in range(NB - 1)
                        ]
                        tail = (
                            xr[: 2 * RTL, (NB - 1) * Wc + m0 : (NB - 1) * Wc + m0 + msz],
                            pk_pr,
                            pk_pi,
                        )
                    else:
                        w_pr = wr + wi
                        r_pr = [
                            gr[:ksz, k * Wc : (k + 1) * Wc]
                            for k, (k0, ksz) in enumerate(BL)
                        ] + [
                            gni[:ksz, k * Wc : (k + 1) * Wc]
                            for k, (k0, ksz) in enumerate(BL)
                        ]
                        w_pi = wr + wi
                        r_pi = [
                            gi[:ksz, k * Wc : (k + 1) * Wc]
                            for k, (k0, ksz) in enumerate(BL)
                        ] + [
                            gr[:ksz, k * Wc : (k + 1) * Wc]
                            for k, (k0, ksz) in enumerate(BL)
                        ]

                    pr = psp.tile([128, Wc], f32, tag="mm")
                    if packed_out and packing and m == NB - 1:
                        # pack the m-tail pair into ONE psum tile: pr rows on
                        # partitions 0:msz, pi rows on 64:64+msz (the PE
                        # writes the upper column group directly - no
                        # partition-shift DMA needed at evacuation)
                        pi = None
                        out_a = pr[:msz, :]
                        out_b = pr[64 : 64 + msz, :]
                    else:
                        pi = psp.tile([128, Wc], f32, tag="mm")
                        out_a = pr[:msz, :]
                        out_b = pi[:msz, :]
                    mm_group2_head(
                        out_a, w_pr, r_pr, out_b, w_pi, r_pi,
                        has_tail=tail is not None,
                    )
                    pending.append((m, msz, pr, pi, tail))
                    import os as _os
                    if len(pending) > (0 if _os.environ.get("NO_DEFER") else 1):
                        flush_one()
                while pending:
                    flush_one()

            def evac_copy(dst_r, dst_i, packed_out, sidx=0):
                def f(m, msz, pr, pi):
                    c0 = m * Wc
                    if pi is None:
                        # packed m-tail pair in one psum tile: both halves
                        # evacuate with partition-aligned engine copies
                        a.copy(out=dst_r[:msz, c0 : c0 + Wc], in_=pr[:msz, :])
                        a.copy(
                            out=dst_r[64 : 64 + msz, c0 : c0 + Wc],
                            in_=pr[64 : 64 + msz, :],
                        )
                        return
                    a.copy(out=dst_r[:msz, c0 : c0 + Wc], in_=pr[:msz, :])
                    a.copy(out=dst_i[:msz, c0 : c0 + Wc], in_=pi[:msz, :])
                return f

            def evac_mask(dst_r, dst_i, sidx=1):
                # must be DVE: only ACT/DVE can access PSUM, and ACT cannot
                # do tensor*tensor; pair-interleaving hides the latency
                def f(m, msz, pr, pi):
                    c0 = m * Wc
                    if pi is None:
                        v.tensor_tensor(
                            out=dst_r[:msz, c0 : c0 + Wc], in0=pr[:msz, :],
                            in1=mpk[:msz, :], op=OP.mult,
                        )
                        v.tensor_tensor(
                            out=dst_r[64 : 64 + msz, c0 : c0 + Wc],
                            in0=pr[64 : 64 + msz, :],
                            in1=mpk[64 : 64 + msz, :], op=OP.mult,
                        )
                        return
                    mk = maskf[:msz, c0 : c0 + Wc]
                    v.tensor_tensor(
                        out=dst_r[:msz, c0 : c0 + Wc], in0=pr[:msz, :], in1=mk,
                        op=OP.mult,
                    )
                    v.tensor_tensor(
                        out=dst_i[:msz, c0 : c0 + Wc], in0=pi[:msz, :], in1=mk,
                        op=OP.mult,
                    )
                return f

            def proj(c):
                """Coil projection cp = p * csm (all-bf16).

                DVE carries 4 ops (2x_1p), Pool 2. Emitted ahead of the
                consuming stages so the elementwise engines compute it while
                PE runs earlier coils' stages.
                """
                slot = c % 2
                csr, csi = cs[c]
                g.tensor_tensor(out=pj1, in0=pb_r, in1=csr, op=OP.mult)
                g.tensor_tensor(out=pj2, in0=pb_i, in1=csi, op=OP.mult)
                v.tensor_tensor(out=cp[slot][0], in0=pj1, in1=pj2, op=OP.subtract)
                v.tensor_tensor(out=pj3, in0=pb_r, in1=csi, op=OP.mult)
                v.tensor_tensor(out=pj4, in0=pb_i, in1=csr, op=OP.mult)
                v.tensor_tensor(out=cp[slot][1], in0=pj3, in1=pj4, op=OP.add)
                if packing and dma_pack:
                    # pack cp: copy im tail rows into cp_r block NB-1 p64:128
                    # (partition-shifting SBUF->SBUF DMA)
                    nc.sync.dma_start(
                        out=cp[slot][0][64 : 64 + RTL, (NB - 1) * Wc : NB * Wc],
                        in_=cp[slot][1][0:RTL, (NB - 1) * Wc : NB * Wc],
                    )

            def stage(stg, c):
                slot = c % 2
                stage1_packed_in = bool(packing and dma_pack)
                if stg == 0:
                    mm_stage(cp[slot][0], cp[slot][1], gF, stage1_packed_in,
                             True, evac_copy(s1[slot][0], s1[slot][1], True, 0))
                elif stg == 1:
                    mm_stage(s1[slot][0], s1[slot][1], gF, True, True,
                             evac_mask(km[slot][0], km[slot][1]))
                elif stg == 2:
                    mm_stage(km[slot][0], km[slot][1], gB, True, True,
                             evac_copy(s3[slot][0], s3[slot][1], True, 2))
                else:
                    mm_stage(s3[slot][0], s3[slot][1], gB, True, False,
                             evac_copy(zz[slot][0], zz[slot][1], False))

            def qaccum(c):
                # q += z * conj(csm); bf16 products/pair-sums on DVE, the
                # fp32 accumulate split DVE (re) / Pool (im)
                slot = c % 2
                csr, csi = cs[c]
                zr, zi = zz[slot]
                v.tensor_tensor(out=tA, in0=zr, in1=csr, op=OP.mult)
                v.tensor_tensor(out=tB, in0=zi, in1=csi, op=OP.mult)
                v.tensor_tensor(out=tC, in0=tA, in1=tB, op=OP.add)
                v.tensor_tensor(out=q_r, in0=q_r, in1=tC, op=OP.add)
                g.tensor_tensor(out=tD, in0=zi, in1=csr, op=OP.mult)
                g.tensor_tensor(out=tE, in0=zr, in1=csi, op=OP.mult)
                v.tensor_tensor(out=tF, in0=tD, in1=tE, op=OP.subtract)
                g.tensor_tensor(out=q_i, in0=q_i, in1=tF, op=OP.add)

            # last-iteration |masked kspace|^2: ACT Square+accum, one
            # partials column per (coil, component)
            def ksq_accum(c):
                slot = c % 2
                kr, ki = km[slot]
                col = 6 + 2 * c
                a.activation(out=tA, in_=kr, func=ACTF.Square,
                             accum_out=partials[:, col : col + 1])
                a.activation(out=tB, in_=ki, func=ACTF.Square,
                             accum_out=partials[:, col + 1 : col + 2])

            for rep in range(reps):
                # ---- (re)init: r = us + mu*rec; p = r; b = 0
                if rep > 0:
                    load_folded(pp[1][0], us[0])
                    load_folded(pp[1][1], us[1])
                    load_folded(b_r, rec[0])
                    load_folded(b_i, rec[1])
                STT_V(out=r_r, in0=b_r, scalar=mu_b[:, :1], in1=pp[1][0],
                      op0=OP.mult, op1=OP.add)
                STT_V(out=r_i, in0=b_i, scalar=mu_b[:, :1], in1=pp[1][1],
                      op0=OP.mult, op1=OP.add)
                a.copy(out=pp[0][0], in_=r_r)
                a.copy(out=pp[0][1], in_=r_i)
                a.copy(out=pb_r, in_=r_r)
                a.copy(out=pb_i, in_=r_i)
                v.memset(b_r, 0.0)
                v.memset(b_i, 0.0)

                for it in range(iters):
                    p_r, p_i = pp[it % 2]
                    pn_r, pn_i = pp[(it + 1) % 2]
                    last = it + 1 == iters and rep + 1 == reps
                    # q = mu * p (coils accumulate on top); ACT scale-copy
                    if not last:
                        a.activation(out=q_r, in_=p_r, func=ACTF.Copy,
                                     scale=mu_b[:, :1])
                        a.activation(out=q_i, in_=p_i, func=ACTF.Copy,
                                     scale=mu_b[:, :1])
                    proj(0)
                    if Cc > 1:
                        proj(1)
                    n_stg = 2 if last else 4
                    # pairwise stage interleave: s1(c) s1(c+1) s2(c) s2(c+1)
                    # ... so every inter-stage evacuation hides behind the
                    # sibling coil's matmuls
                    for c in range(0, Cc, 2):
                        cset = [c] + ([c + 1] if c + 1 < Cc else [])
                        for stg in range(n_stg):
                            for cc in cset:
                                stage(stg, cc)
                                if stg == 3:
                                    qaccum(cc)
                                if last and stg == 1:
                                    ksq_accum(cc)
                            if stg == 0:
                                # next pair's projections (cp slots freed by
                                # this pair's stage-1 reads)
                                for cc in cset:
                                    if cc + 2 < Cc:
                                        proj(cc + 2)
                    if last:
                        # pq = sum_c ||M F C_c p||^2 + mu ||p||^2
                        pn0 = 6 + 2 * Cc
                        a.activation(out=dump, in_=p_r, func=ACTF.Square,
                                     accum_out=partials[:, pn0 : pn0 + 1])
                        a.activation(out=dump, in_=p_i, func=ACTF.Square,
                                     accum_out=partials[:, pn0 + 1 : pn0 + 2])
                        k = pn0 + 2
                        ps1 = psp.tile([1, NPART], f32, tag="mm")
                        nc.tensor.matmul(ps1[:1, :k], lhsT=ones_col[:, :1],
                                         rhs=partials[:, :k], start=True,
                                         stop=True)
                        a.copy(out=redsums[:1, :k], in_=ps1[:1, :k])
                        # sum the 2C per-coil kspace columns in one reduce
                        v.reduce_sum(out=scl[:1, 0:1],
                                     in_=redsums[:1, 6 : 6 + 2 * Cc],
                                     axis=mybir.AxisListType.X)
                        TT(out=scl[:1, 1:2], in0=redsums[:1, pn0 : pn0 + 1],
                           in1=redsums[:1, pn0 + 1 : pn0 + 2], op=OP.add)
                        # pq = ksq + mu * pnorm
                        STT_V(out=scl[:1, 2:3], in0=scl[:1, 1:2],
                              scalar=mu_sb[:1, :1], in1=scl[:1, 0:1],
                              op0=OP.mult, op1=OP.add)
                        v.reciprocal(out=scl[:1, 3:4], in_=scl[:1, 2:3])
                        TT(out=alphas[:1, 0:1], in0=rr_t[:1, :1],
                           in1=scl[:1, 3:4], op=OP.mult)      # alpha = rr/pq
                        psb2 = psp.tile([128, 16], f32, tag="mm")
                        nc.tensor.matmul(psb2[:, :1], lhsT=ones_row[:1, :128],
                                         rhs=alphas[:1, :1], start=True,
                                         stop=True)
                        a.copy(out=bc[:, :1], in_=psb2[:, :1])
                        a_ = bc[:, 0:1]
                        STT_V(out=b_r, in0=p_r, scalar=a_, in1=b_r,
                              op0=OP.mult, op1=OP.add)
                        STT_V(out=b_i, in0=p_i, scalar=a_, in1=b_i,
                              op0=OP.mult, op1=OP.add)
                        continue

                    # ---- per-sample scalars: pq (DVE mult+reduce), qq (ACT
                    # square+accum), and rr on iter 0
                    TT(out=dump, in0=q_r, in1=p_r, op=OP.mult)
                    v.reduce_sum(out=partials[:, 0:1], in_=dump,
                                 axis=mybir.AxisListType.X)
                    TT(out=tC, in0=q_i, in1=p_i, op=OP.mult)
                    v.reduce_sum(out=partials[:, 1:2], in_=tC,
                                 axis=mybir.AxisListType.X)
                    a.activation(out=tA, in_=q_r, func=ACTF.Square,
                                 accum_out=partials[:, 2:3])
                    a.activation(out=tB, in_=q_i, func=ACTF.Square,
                                 accum_out=partials[:, 3:4])
                    k = 4
                    if it == 0:
                        a.activation(out=tD, in_=r_r, func=ACTF.Square,
                                     accum_out=partials[:, 4:5])
                        a.activation(out=tE, in_=r_i, func=ACTF.Square,
                                     accum_out=partials[:, 5:6])
                        k = 6
                    ps1 = psp.tile([1, 16], f32, tag="mm")
                    nc.tensor.matmul(ps1[:1, :k], lhsT=ones_col[:, :1],
                                     rhs=partials[:, :k], start=True, stop=True)
                    a.copy(out=redsums[:1, :k], in_=ps1[:1, :k])
                    # pq = c0+c1, qq = c2+c3 (, rr = c4+c5)
                    TT(out=scl[:1, 0:1], in0=redsums[:1, 0:1],
                       in1=redsums[:1, 1:2], op=OP.add)
                    TT(out=scl[:1, 1:2], in0=redsums[:1, 2:3],
                       in1=redsums[:1, 3:4], op=OP.add)
                    if it == 0:
                        TT(out=rr_t[:1, :1], in0=redsums[:1, 4:5],
                           in1=redsums[:1, 5:6], op=OP.add)
                    v.reciprocal(out=scl[:1, 2:3], in_=scl[:1, 0:1])
                    TT(out=alphas[:1, 0:1], in0=rr_t[:1, :1],
                       in1=scl[:1, 2:3], op=OP.mult)          # alpha = rr/pq
                    TT(out=scl[:1, 3:4], in0=alphas[:1, 0:1],
                       in1=alphas[:1, 0:1], op=OP.mult)       # alpha^2
                    TT(out=scl[:1, 4:5], in0=scl[:1, 3:4],
                       in1=scl[:1, 1:2], op=OP.mult)          # alpha^2 qq
                    TT(out=rrn_t[:1, :1], in0=scl[:1, 4:5],
                       in1=rr_t[:1, :1], op=OP.subtract)      # rr_new
                    v.reciprocal(out=scl[:1, 5:6], in_=rr_t[:1, :1])
                    TT(out=alphas[:1, 2:3], in0=rrn_t[:1, :1],
                       in1=scl[:1, 5:6], op=OP.mult)          # beta
                    v.tensor_scalar_mul(out=alphas[:1, 1:2],
                                        in0=alphas[:1, 0:1], scalar1=-1.0)
                    a.copy(out=rr_t[:1, :1], in_=rrn_t[:1, :1])
                    psb2 = psp.tile([128, 16], f32, tag="mm")
                    nc.tensor.matmul(psb2[:, :3], lhsT=ones_row[:1, :128],
                                     rhs=alphas[:1, :3], start=True, stop=True)
                    a.copy(out=bc[:, :3], in_=psb2[:, :3])
                    a_ = bc[:, 0:1]
                    na = bc[:, 1:2]
                    bet = bc[:, 2:3]
                    # r -= alpha q ; p' = r + beta p ; b += alpha p (reads old
                    # p, emitted last - it doesn't gate the next iteration).
                    STT_V(out=r_r, in0=q_r, scalar=na, in1=r_r,
                          op0=OP.mult, op1=OP.add)
                    STT_V(out=pn_r, in0=p_r, scalar=bet, in1=r_r,
                          op0=OP.mult, op1=OP.add)
                    a.copy(out=pb_r, in_=pn_r)
                    STT_V(out=r_i, in0=q_i, scalar=na, in1=r_i,
                          op0=OP.mult, op1=OP.add)
                    STT_V(out=pn_i, in0=p_i, scalar=bet, in1=r_i,
                          op0=OP.mult, op1=OP.add)
                    a.copy(out=pb_i, in_=pn_i)
                    STT_V(out=b_r, in0=p_r, scalar=a_, in1=b_r,
                          op0=OP.mult, op1=OP.add)
                    STT_V(out=b_i, in0=p_i, scalar=a_, in1=b_i,
                          op0=OP.mult, op1=OP.add)

            store_folded(b_r, out_d[0])
            store_folded(b_i, out_d[1])

    nc.compile()
    return nc


def _host_inputs(Hc, Wc, packing, RTL, NB):
    bf = np.float16
    Fc = _centered_dft(Hc)
    f_r = np.ascontiguousarray(Fc.real).astype(np.float32)
    f_i = np.ascontiguousarray(Fc.imag).astype(np.float32)
    f_ni = (-f_i).astype(np.float32)
    shared = {"f_r": f_r.astype(bf), "f_i": f_i.astype(bf), "f_ni": f_ni.astype(bf)}
    if packing:
        t0 = 128 * (NB - 1)
        fr2 = f_r[t0:Hc, :]
        fi2 = f_i[t0:Hc, :]
        fni2 = f_ni[t0:Hc, :]
        fpk = np.concatenate(
            [
                np.concatenate([fr2, fni2], axis=0),
                np.concatenate([fi2, fr2], axis=0),
                np.concatenate([fr2, fi2], axis=0),
                np.concatenate([fni2, fr2], axis=0),
            ],
            axis=1,
        )
        shared["f_pk"] = np.ascontiguousarray(fpk).astype(bf)
    return shared


def prepare(us_image, reconstruction, mask, csm_r, csm_i, mu, reps=1):
    """Build (cached) the Bass module and per-core input maps."""
    bf = np.float16
    Bc, _, Hc, Wc = us_image.shape
    Cc = csm_r.shape[1]
    n_cores = Bc
    iters = CG_ITER

    BL = _blocks(Hc)
    NB = len(BL)
    RTL = BL[-1][1] if BL[-1][1] < 128 else 0
    packing = RTL > 0 and 2 * RTL <= 128

    key = (Hc, Wc, Cc, iters, n_cores, reps)
    if key not in _nc_cache:
        _nc_cache[key] = _build(Hc, Wc, Cc, iters, n_cores, reps=reps)
    nc = _nc_cache[key]

    shared = _host_inputs(Hc, Wc, packing, RTL, NB)

    in_maps = []
    for b in range(n_cores):
        m = {
            "us_image": np.ascontiguousarray(us_image[b], dtype=np.float32),
            "reconstruction": np.ascontiguousarray(
                reconstruction[b], dtype=np.float32
            ),
            "mask": np.ascontiguousarray(mask[b, 0]).astype(bf),
            "csm_r": np.ascontiguousarray(csm_r[b]).astype(bf),
            "csm_i": np.ascontiguousarray(csm_i[b]).astype(bf),
            "mu": np.ascontiguousarray(mu, dtype=np.float32),
        }
        m.update(shared)
        if packing:
            t0 = 128 * (NB - 1)
            m2 = np.ascontiguousarray(mask[b, 0, t0:Hc, :])
            m["mask_pk"] = np.concatenate([m2, m2], axis=0).astype(bf)
        in_maps.append(m)
    return in_maps, nc, n_cores


def kernel(us_image, reconstruction, mask, csm_r, csm_i, mu):
    global LAST_RESULT
    from concourse.bass_utils import run_bass_kernel_spmd

    in_maps, nc, n_cores = prepare(us_image, reconstruction, mask, csm_r, csm_i, mu)
    res = run_bass_kernel_spmd(nc, in_maps, core_ids=list(range(n_cores)))
    LAST_RESULT = res
    out = np.stack([res.results[b]["out"] for b in range(n_cores)], axis=0)
    return out.astype(np.float32)


# revision 19
# speedup vs baseline: 1.0924x; 1.0924x over previous
"""MRI data-consistency CG solver on 8 Trainium2 NeuronCores.

Sharding: pure data-parallel, 1 batch sample per core. The CG alpha/beta
scalars are computed per-sample (deviation from the reference's global
batch sums is ~3e-4 relative, far below tolerance), so cores run fully
independently - no collectives.

Per coil, the centered 2D FFT / IFFT are chained PE matmuls with the
centered DFT matrix Fc (Fc = S F S is symmetric, so
  stage1 = X^T Fc   (data as lhsT; output transposed)
  stage2 = stage1^T Fc = Fc X Fc  (natural orientation again)
-> no explicit transposes anywhere).

v2 data-path: the matmul STATIONARY side (the data: cp/s1/km/s3) is
bf16 - LDWEIGHTS then streams 2-byte rows and fully hides under the
320-col f32r moving operand (G matrices stay f32r => fp32-grade DFT
coefficients; fp32 PSUM accumulate). All csm-coupled elementwise ops
run in bf16 on DVE (2x_1p mode) with the fp32 CG state (r/p/q/b) kept
in fp32. Measured-model final rel err ~5e-3 vs 2e-2 gate.

All 32 csm half-tiles are SBUF-resident in bf16 (no per-iteration
re-streaming). The CG scalars use fused tensor_tensor_reduce (pq) and
ACT Square+accum (qq/rr). The last CG iteration only needs
alpha = rr / p^H A p, and p^H A p = sum_c ||M F C_c p||^2 + mu ||p||^2,
so stages 3/4 + the coil combine are skipped there; the masked-kspace
norms accumulate via chained TTRs.

Field layout: each 320x320 field lives in one SBUF tile [128, 3*320]
("folded"): block b (cols [b*320,(b+1)*320)) holds rows [b*128, ...) of
the matrix. Block 2 only uses partitions 0..63 in the standard layout
(junk kept at 0); the matmul-chain intermediates instead use a "packed"
layout where block 2 of the REAL tile holds [re rows 256:320 (p0:64);
im rows 256:320 (p64:128)], which lets the two 64-row contraction tails
merge into one full 128-row matmul (k2-packing).

CG scalar algebra: alpha is real (p^H A p real) and r^H q == p^H q by
A-conjugacy, so per iteration only two sums are needed:
  pq = sum(q_r p_r + q_i p_i),  qq = sum(|q|^2)
  alpha = rr/pq;  rr_new = alpha^2 qq - rr;  beta = rr_new/rr
"""

import numpy as np

CG_ITER = 10

_nc_cache = {}
LAST_RESULT = None


def _blocks(n):
    out = []
    r0 = 0
    while r0 < n:
        sz = min(128, n - r0)
        out.append((r0, sz))
        r0 += sz
    return out


def _centered_dft(n):
    # Columns of Fc = centered orthonormal DFT applied to unit vectors:
    # y = fftshift(fft(ifftshift(x))) = Fc @ x. Fc is symmetric for even n.
    eye = np.eye(n)
    Fc = np.fft.fftshift(
        np.fft.fft(np.fft.ifftshift(eye, axes=0), axis=0, norm="ortho"), axes=0
    )
    return Fc


def _build(Hc, Wc, Cc, iters, n_cores, reps=1, dma_pack=True):
    import concourse.bacc as bacc
    import concourse.mybir as mybir
    import concourse.tile as tile

    f32 = mybir.dt.float32
    f32r = mybir.dt.float32r
    bf16 = mybir.dt.float16  # "bf16" name kept; fp16 = same speed, 4 more mantissa bits
    OP = mybir.AluOpType
    ACTF = mybir.ActivationFunctionType

    nc = bacc.Bacc(trn_type="TRN2", num_devices=n_cores)

    BL = _blocks(Hc)
    NB = len(BL)
    FW = NB * Wc
    # size of the partial tail block (0 if H divides evenly)
    RTL = BL[-1][1] if BL[-1][1] < 128 else 0
    packing = RTL > 0 and 2 * RTL <= 128
    import os as _os
    if _os.environ.get("NO_PACK"):
        packing = False

    us = nc.dram_tensor("us_image", [2, Hc, Wc], f32, kind="ExternalInput")
    rec = nc.dram_tensor("reconstruction", [2, Hc, Wc], f32, kind="ExternalInput")
    mask_d = nc.dram_tensor("mask", [Hc, Wc], bf16, kind="ExternalInput")
    csm_r_d = nc.dram_tensor("csm_r", [Cc, Hc, Wc], bf16, kind="ExternalInput")
    csm_i_d = nc.dram_tensor("csm_i", [Cc, Hc, Wc], bf16, kind="ExternalInput")
    mu_d = nc.dram_tensor("mu", [1], f32, kind="ExternalInput")
    fr_d = nc.dram_tensor("f_r", [Hc, Hc], bf16, kind="ExternalInput")
    fi_d = nc.dram_tensor("f_i", [Hc, Hc], bf16, kind="ExternalInput")
    fni_d = nc.dram_tensor("f_ni", [Hc, Hc], bf16, kind="ExternalInput")
    if packing:
        # packed k2 rhs tiles: [Ga[tail rows] on p0:R ; Gb[tail] on pR:2R]
        # slots: 0=[fr;fni] 1=[fi;fr] 2=[fr;fi] 3=[fni;fr]
        fpk_d = nc.dram_tensor("f_pk", [2 * RTL, 4 * Wc], bf16, kind="ExternalInput")
        mpk_d = nc.dram_tensor("mask_pk", [2 * RTL, Wc], bf16, kind="ExternalInput")
    out_d = nc.dram_tensor("out", [2, Hc, Wc], f32, kind="ExternalOutput")

    with tile.TileContext(nc) as tc:
        with (
            tc.tile_pool(name="consts", bufs=1) as consts,
            tc.tile_pool(name="state", bufs=1) as state,
            tc.tile_pool(name="work", bufs=1) as work,
            tc.tile_pool(name="small", bufs=1) as small,
            tc.tile_pool(name="psum", bufs=8, space="PSUM") as psp,
        ):
            zero_f32 = []   # [128, FW] f32 tiles to memset (junk must be 0)
            zero_bf16 = []  # bf16 tiles whose junk feeds reductions

            def T(pool, name, shape, dtype=f32, zero=None):
                tl = pool.tile(shape, dtype, tag=name)
                if zero is None:
                    zero = list(shape) == [128, FW] and dtype == f32
                if zero:
                    (zero_f32 if dtype == f32 else zero_bf16).append(tl)
                return tl

            fr = T(consts, "fr", [128, FW], bf16)
            fi = T(consts, "fi", [128, FW], bf16)
            fni = T(consts, "fni", [128, FW], bf16)
            maskf = T(consts, "maskf", [128, FW], bf16)
            if packing:
                fpk = T(consts, "fpk", [2 * RTL, 4 * Wc], bf16)
                mpk = T(consts, "mpk", [2 * RTL, Wc], bf16)
            ones_col = T(consts, "ones_col", [128, 1])
            ones_row = T(consts, "ones_row", [1, 128])
            mu_b = T(consts, "mu_b", [128, 1])
            mu_sb = T(consts, "mu_sb", [1, 1])

            pp = [
                [T(state, "pA_r", [128, FW]), T(state, "pA_i", [128, FW])],
                [T(state, "pB_r", [128, FW]), T(state, "pB_i", [128, FW])],
            ]
            r_r = T(state, "r_r", [128, FW])
            r_i = T(state, "r_i", [128, FW])
            b_r = T(state, "b_r", [128, FW])
            b_i = T(state, "b_i", [128, FW])
            q_r = T(state, "q_r", [128, FW])
            q_i = T(state, "q_i", [128, FW])
            # bf16 shadow of p for the csm products
            pb_r = T(state, "pb_r", [128, FW], bf16, zero=True)
            pb_i = T(state, "pb_i", [128, FW], bf16, zero=True)

            cp = [[T(work, f"cp_{x}{j}", [128, FW], bf16) for x in "ri"] for j in (0, 1)]
            s1 = [[T(work, f"s1_{x}{j}", [128, FW], bf16) for x in "ri"] for j in (0, 1)]
            # km feeds full-tile TTR reductions on the last iteration
            km = [[T(work, f"km_{x}{j}", [128, FW], bf16, zero=True) for x in "ri"]
                  for j in (0, 1)]
            s3 = [[T(work, f"s3_{x}{j}", [128, FW], bf16) for x in "ri"] for j in (0, 1)]
            # zz feeds the fp32 q accumulation: junk must be 0
            zz = [[T(work, f"zz_{x}{j}", [128, FW], bf16, zero=True) for x in "ri"]
                  for j in (0, 1)]
            # all csm tiles SBUF-resident in bf16
            cs = [
                [T(work, f"cs_{x}{j}", [128, FW], bf16, zero=True) for x in "ri"]
                for j in range(Cc)
            ]
            pj1 = T(work, "pj1", [128, FW], bf16)
            pj2 = T(work, "pj2", [128, FW], bf16)
            pj3 = T(work, "pj3", [128, FW], bf16)
            pj4 = T(work, "pj4", [128, FW], bf16)
            tA = T(work, "tA", [128, FW], bf16)
            tB = T(work, "tB", [128, FW], bf16)
            tC = T(work, "tC", [128, FW], bf16)
            tD = T(work, "tD", [128, FW], bf16)
            tE = T(work, "tE", [128, FW], bf16)
            tF = T(work, "tF", [128, FW], bf16)
            dump = T(work, "dump", [128, FW], bf16)

            # cols: 0/1 pq parts, 2/3 qq parts, 4/5 rr parts,
            # 6..6+2C last-iter |masked ksp|^2 per coil, then 2 p-norm cols
            NPART = 8 + 2 * Cc
            partials = T(small, "partials", [128, NPART])
            redsums = T(small, "redsums", [1, NPART])
            scl = T(small, "scl", [1, 8])
            alphas = T(small, "alphas", [1, 4])
            bc = T(small, "bc", [128, 4])
            rr_t = T(small, "rr", [1, 1])
            rrn_t = T(small, "rrn", [1, 1])

            v = nc.vector
            g = nc.gpsimd
            a = nc.scalar
            STT_V = v.scalar_tensor_tensor
            TT = v.tensor_tensor

            # ---- init: zero everything (keeps junk regions at 0)
            for tl in zero_f32 + zero_bf16:
                v.memset(tl, 0.0)
            v.memset(partials, 0.0)
            v.memset(ones_col, 1.0)
            v.memset(ones_row, 1.0)

            def load_folded(dst, src2d):
                nbf = Hc // 128
                full = nbf * 128
                if nbf:
                    nc.sync.dma_start(
                        out=dst[:, 0 : nbf * Wc].rearrange("p (b w) -> p b w", b=nbf),
                        in_=src2d[0:full, :].rearrange("(b p) w -> p b w", p=128),
                    )
                if full < Hc:
                    rem = Hc - full
                    nc.sync.dma_start(
                        out=dst[:rem, nbf * Wc : (nbf + 1) * Wc],
                        in_=src2d[full:Hc, :],
                    )

            def store_folded(src, dst2d):
                nbf = Hc // 128
                full = nbf * 128
                if nbf:
                    nc.sync.dma_start(
                        out=dst2d[0:full, :].rearrange("(b p) w -> p b w", p=128),
                        in_=src[:, 0 : nbf * Wc].rearrange("p (b w) -> p b w", b=nbf),
                    )
                if full < Hc:
                    rem = Hc - full
                    nc.sync.dma_start(
                        out=dst2d[full:Hc, :],
                        in_=src[:rem, nbf * Wc : (nbf + 1) * Wc],
                    )

            # order: init-chain dependencies first (mu, us/rec for r, csm0/1
            # for the first projections), then the DFT matrices needed by
            # stage 1, then the rest
            nc.sync.dma_start(out=mu_sb[:1, :1], in_=mu_d[None, :])
            # borrow pp[1] and b for the r-init staging (all junk pre-zeroed)
            load_folded(pp[1][0], us[0])
            load_folded(pp[1][1], us[1])
            load_folded(b_r, rec[0])
            load_folded(b_i, rec[1])

            psb = psp.tile([128, 16], f32, tag="mm")
            nc.tensor.matmul(
                psb[:, :1], lhsT=ones_row[:1, :128], rhs=mu_sb[:1, :1],
                start=True, stop=True,
            )
            a.copy(out=mu_b[:, :1], in_=psb[:, :1])

            def load_csm(ci_):
                load_folded(cs[ci_][0], csm_r_d[ci_])
                load_folded(cs[ci_][1], csm_i_d[ci_])

            for j in range(min(Cc, 2)):
                load_csm(j)
            load_folded(fr, fr_d[:])
            load_folded(fi, fi_d[:])
            load_folded(fni, fni_d[:])
            if packing:
                nc.sync.dma_start(out=fpk, in_=fpk_d[:])
                nc.sync.dma_start(out=mpk, in_=mpk_d[:])
            load_folded(maskf, mask_d[:])
            for j in range(2, Cc):
                load_csm(j)

            # G-sets: (gr, gi, gni, pk_pr, pk_pi)
            if packing:
                gF = (fr, fi, fni, fpk[:, 0:Wc], fpk[:, Wc : 2 * Wc])
                gB = (fr, fni, fi, fpk[:, 2 * Wc : 3 * Wc], fpk[:, 3 * Wc : 4 * Wc])
            else:
                gF = (fr, fi, fni, None, None)
                gB = (fr, fni, fi, None, None)

            def mm_group2_head(out_a, wa, ra, out_b, wb, rb, has_tail):
                """Interleaved head matmuls of two accumulation groups.

                If has_tail, groups are left open for a deferred k2 pair.
                """
                n = len(wa)
                assert len(wb) == n
                for j in range(n):
                    stop = (not has_tail) and j == n - 1
                    nc.tensor.matmul(
                        out_a, lhsT=wa[j], rhs=ra[j], start=(j == 0), stop=stop,
                    )
                    nc.tensor.matmul(
                        out_b, lhsT=wb[j], rhs=rb[j], start=(j == 0), stop=stop,
                    )

            def mm_stage(xr, xi, gset, packed_in, packed_out, consume):
                """out = (xr + i xi)^T @ (gr + i gi); gni = -gi precomputed.

                packed_in: xr block NB-1 holds [re_tail; im_tail] (k2-packing)
                packed_out: m-tail block's imag half staged for partition shift
                """
                gr, gi, gni, pk_pr, pk_pi = gset

                def wslices(tl, m0, msz):
                    # weight slices per k-block: list of (ap, rhs) pairs
                    return [
                        tl[:ksz, k * Wc + m0 : k * Wc + m0 + msz]
                        for k, (k0, ksz) in enumerate(BL)
                    ]

                pending = []

                def flush_one():
                    # emit the deferred k2 pair of the oldest open m-block,
                    # close its groups, and evacuate
                    m, msz, pr, pi, tail = pending.pop(0)
                    if tail is not None:
                        (wk2, rk2_a, rk2_b) = tail
                        out_b = pi[:msz, :] if pi is not None else pr[64 : 64 + msz, :]
                        nc.tensor.matmul(
                            pr[:msz, :], lhsT=wk2, rhs=rk2_a,
                            start=False, stop=True,
                        )
                        nc.tensor.matmul(
                            out_b, lhsT=wk2, rhs=rk2_b,
                            start=False, stop=True,
                        )
                    consume(m, msz, pr, pi)

                for m in range(NB):
                    m0, msz = BL[m]
                    wr = wslices(xr, m0, msz)
                    wi = wslices(xi, m0, msz)
                    tail = None
                    if packed_in and packing:
                        # last k-block: single 128-row packed MM (deferred -
                        # its input is written by the previous stage's
                        # partition-shift, so give it pipeline slack)
                        w_pr = wr[:-1] + wi[:-1]
                        r_pr = [
                            gr[:128, k * Wc : (k + 1) * Wc] for k in range(NB - 1)
                        ] + [
                            gni[:128, k * Wc : (k + 1) * Wc] for k in range(NB - 1)
                        ]
                        w_pi = wr[:-1] + wi[:-1]
                        r_pi = [
                            gi[:128, k * Wc : (k + 1) * Wc] for k in range(NB - 1)
                        ] + [
                            gr[:128, k * Wc : (k + 1) * Wc] for k in range(NB - 1)
                        ]
                        tail = (
                            xr[: 2 * RTL, (NB - 1) * Wc + m0 : (NB - 1) * Wc + m0 + msz],
                            pk_pr,
                            pk_pi,
                        )
                    else:
                        w_pr = wr + wi
                        r_pr = [
                            gr[:ksz, k * Wc : (k + 1) * Wc]
                            for k, (k0, ksz) in enumerate(BL)
                        ] + [
                            gni[:ksz, k * Wc : (k + 1) * Wc]
                            for k, (k0, ksz) in enumerate(BL)
                        ]
                        w_pi = wr + wi
                        r_pi = [
                            gi[:ksz, k * Wc : (k + 1) * Wc]
                            for k, (k0, ksz) in enumerate(BL)
                        ] + [
                            gr[:ksz, k * Wc : (k + 1) * Wc]
                            for k, (k0, ksz) in enumerate(BL)
                        ]

                    pr = psp.tile([128, Wc], f32, tag="mm")
                    if packed_out and packing and m == NB - 1:
                        # pack the m-tail pair into ONE psum tile: pr rows on
                        # partitions 0:msz, pi rows on 64:64+msz (the PE
                        # writes the upper column group directly - no
                        # partition-shift DMA needed at evacuation)
                        pi = None
                        out_a = pr[:msz, :]
                        out_b = pr[64 : 64 + msz, :]
                    else:
                        pi = psp.tile([128, Wc], f32, tag="mm")
                        out_a = pr[:msz, :]
                        out_b = pi[:msz, :]
                    mm_group2_head(
                        out_a, w_pr, r_pr, out_b, w_pi, r_pi,
                        has_tail=tail is not None,
                    )
                    pending.append((m, msz, pr, pi, tail))
                    import os as _os
                    if len(pending) > (0 if _os.environ.get("NO_DEFER") else 1):
                        flush_one()
                while pending:
                    flush_one()

            def evac_copy(dst_r, dst_i, packed_out, sidx=0):
                def f(m, msz, pr, pi):
                    c0 = m * Wc
                    if pi is None:
                        # packed m-tail pair in one psum tile: both halves
                        # evacuate with partition-aligned engine copies
                        a.copy(out=dst_r[:msz, c0 : c0 + Wc], in_=pr[:msz, :])
                        a.copy(
                            out=dst_r[64 : 64 + msz, c0 : c0 + Wc],
                            in_=pr[64 : 64 + msz, :],
                        )
                        return
                    a.copy(out=dst_r[:msz, c0 : c0 + Wc], in_=pr[:msz, :])
                    a.copy(out=dst_i[:msz, c0 : c0 + Wc], in_=pi[:msz, :])
                return f

            def evac_mask(dst_r, dst_i, sidx=1):
                # must be DVE: only ACT/DVE can access PSUM, and ACT cannot
                # do tensor*tensor; pair-interleaving hides the latency
                def f(m, msz, pr, pi):
                    c0 = m * Wc
                    if pi is None:
                        v.tensor_tensor(
                            out=dst_r[:msz, c0 : c0 + Wc], in0=pr[:msz, :],
                            in1=mpk[:msz, :], op=OP.mult,
                        )
                        v.tensor_tensor(
                            out=dst_r[64 : 64 + msz, c0 : c0 + Wc],
                            in0=pr[64 : 64 + msz, :],
                            in1=mpk[64 : 64 + msz, :], op=OP.mult,
                        )
                        return
                    mk = maskf[:msz, c0 : c0 + Wc]
                    v.tensor_tensor(
                        out=dst_r[:msz, c0 : c0 + Wc], in0=pr[:msz, :], in1=mk,
                        op=OP.mult,
                    )
                    v.tensor_tensor(
                        out=dst_i[:msz, c0 : c0 + Wc], in0=pi[:msz, :], in1=mk,
                        op=OP.mult,
                    )
                return f

            def proj(c):
                """Coil projection cp = p * csm (all-bf16).

                DVE carries 4 ops (2x_1p), Pool 2. Emitted ahead of the
                consuming stages so the elementwise engines compute it while
                PE runs earlier coils' stages.
                """
                slot = c % 2
                csr, csi = cs[c]
                g.tensor_tensor(out=pj1, in0=pb_r, in1=csr, op=OP.mult)
                g.tensor_tensor(out=pj2, in0=pb_i, in1=csi, op=OP.mult)
                v.tensor_tensor(out=cp[slot][0], in0=pj1, in1=pj2, op=OP.subtract)
                v.tensor_tensor(out=pj3, in0=pb_r, in1=csi, op=OP.mult)
                v.tensor_tensor(out=pj4, in0=pb_i, in1=csr, op=OP.mult)
                v.tensor_tensor(out=cp[slot][1], in0=pj3, in1=pj4, op=OP.add)
                if packing and dma_pack:
                    # pack cp: copy im tail rows into cp_r block NB-1 p64:128
                    # (partition-shifting SBUF->SBUF DMA)
                    nc.sync.dma_start(
                        out=cp[slot][0][64 : 64 + RTL, (NB - 1) * Wc : NB * Wc],
                        in_=cp[slot][1][0:RTL, (NB - 1) * Wc : NB * Wc],
                    )

            def stage(stg, c):
                slot = c % 2
                stage1_packed_in = bool(packing and dma_pack)
                if stg == 0:
                    mm_stage(cp[slot][0], cp[slot][1], gF, stage1_packed_in,
                             True, evac_copy(s1[slot][0], s1[slot][1], True, 0))
                elif stg == 1:
                    mm_stage(s1[slot][0], s1[slot][1], gF, True, True,
                             evac_mask(km[slot][0], km[slot][1]))
                elif stg == 2:
                    mm_stage(km[slot][0], km[slot][1], gB, True, True,
                             evac_copy(s3[slot][0], s3[slot][1], True, 2))
                else:
                    mm_stage(s3[slot][0], s3[slot][1], gB, True, False,
                             evac_copy(zz[slot][0], zz[slot][1], False))

            def qaccum(c):
                # q += z * conj(csm); bf16 products/pair-sums on DVE, the
                # fp32 accumulate split DVE (re) / Pool (im)
                slot = c % 2
                csr, csi = cs[c]
                zr, zi = zz[slot]
                v.tensor_tensor(out=tA, in0=zr, in1=csr, op=OP.mult)
                v.tensor_tensor(out=tB, in0=zi, in1=csi, op=OP.mult)
                v.tensor_tensor(out=tC, in0=tA, in1=tB, op=OP.add)
                v.tensor_tensor(out=q_r, in0=q_r, in1=tC, op=OP.add)
                v.tensor_tensor(out=tD, in0=zi, in1=csr, op=OP.mult)
                v.tensor_tensor(out=tE, in0=zr, in1=csi, op=OP.mult)
                v.tensor_tensor(out=tF, in0=tD, in1=tE, op=OP.subtract)
                g.tensor_tensor(out=q_i, in0=q_i, in1=tF, op=OP.add)

            # last-iteration |masked kspace|^2: ACT Square+accum, one
            # partials column per (coil, component)
            def ksq_accum(c):
                slot = c % 2
                kr, ki = km[slot]
                col = 6 + 2 * c
                a.activation(out=tA, in_=kr, func=ACTF.Square,
                             accum_out=partials[:, col : col + 1])
                a.activation(out=tB, in_=ki, func=ACTF.Square,
                             accum_out=partials[:, col + 1 : col + 2])

            for rep in range(reps):
                # ---- (re)init: r = us + mu*rec; p = r; b = 0
                if rep > 0:
                    load_folded(pp[1][0], us[0])
                    load_folded(pp[1][1], us[1])
                    load_folded(b_r, rec[0])
                    load_folded(b_i, rec[1])
                STT_V(out=r_r, in0=b_r, scalar=mu_b[:, :1], in1=pp[1][0],
                      op0=OP.mult, op1=OP.add)
                STT_V(out=r_i, in0=b_i, scalar=mu_b[:, :1], in1=pp[1][1],
                      op0=OP.mult, op1=OP.add)
                a.copy(out=pp[0][0], in_=r_r)
                a.copy(out=pp[0][1], in_=r_i)
                a.copy(out=pb_r, in_=r_r)
                a.copy(out=pb_i, in_=r_i)
                v.memset(b_r, 0.0)
                v.memset(b_i, 0.0)

                for it in range(iters):
                    p_r, p_i = pp[it % 2]
                    pn_r, pn_i = pp[(it + 1) % 2]
                    last = it + 1 == iters and rep + 1 == reps
                    # q = mu * p (coils accumulate on top); ACT scale-copy
                    if not last:
                        a.activation(out=q_r, in_=p_r, func=ACTF.Copy,
                                     scale=mu_b[:, :1])
                        a.activation(out=q_i, in_=p_i, func=ACTF.Copy,
                                     scale=mu_b[:, :1])
                    proj(0)
                    if Cc > 1:
                        proj(1)
                    n_stg = 2 if last else 4
                    # pairwise stage interleave: s1(c) s1(c+1) s2(c) s2(c+1)
                    # ... so every inter-stage evacuation hides behind the
                    # sibling coil's matmuls
                    for c in range(0, Cc, 2):
                        cset = [c] + ([c + 1] if c + 1 < Cc else [])
                        for stg in range(n_stg):
                            for cc in cset:
                                stage(stg, cc)
                                if stg == 3:
                                    qaccum(cc)
                                if last and stg == 1:
                                    ksq_accum(cc)
                            if stg == 0:
                                # next pair's projections (cp slots freed by
                                # this pair's stage-1 reads)
                                for cc in cset:
                                    if cc + 2 < Cc:
                                        proj(cc + 2)
                    if last:
                        # pq = sum_c ||M F C_c p||^2 + mu ||p||^2
                        pn0 = 6 + 2 * Cc
                        a.activation(out=dump, in_=p_r, func=ACTF.Square,
                                     accum_out=partials[:, pn0 : pn0 + 1])
                        a.activation(out=dump, in_=p_i, func=ACTF.Square,
                                     accum_out=partials[:, pn0 + 1 : pn0 + 2])
                        k = pn0 + 2
                        ps1 = psp.tile([1, NPART], f32, tag="mm")
                        nc.tensor.matmul(ps1[:1, :k], lhsT=ones_col[:, :1],
                                         rhs=partials[:, :k], start=True,
                                         stop=True)
                        a.copy(out=redsums[:1, :k], in_=ps1[:1, :k])
                        # sum the 2C per-coil kspace columns in one reduce
                        v.reduce_sum(out=scl[:1, 0:1],
                                     in_=redsums[:1, 6 : 6 + 2 * Cc],
                                     axis=mybir.AxisListType.X)
                        TT(out=scl[:1, 1:2], in0=redsums[:1, pn0 : pn0 + 1],
                           in1=redsums[:1, pn0 + 1 : pn0 + 2], op=OP.add)
                        # pq = ksq + mu * pnorm
                        STT_V(out=scl[:1, 2:3], in0=scl[:1, 1:2],
                              scalar=mu_sb[:1, :1], in1=scl[:1, 0:1],
                              op0=OP.mult, op1=OP.add)
                        v.reciprocal(out=scl[:1, 3:4], in_=scl[:1, 2:3])
                        TT(out=alphas[:1, 0:1], in0=rr_t[:1, :1],
                           in1=scl[:1, 3:4], op=OP.mult)      # alpha = rr/pq
                        psb2 = psp.tile([128, 16], f32, tag="mm")
                        nc.tensor.matmul(psb2[:, :1], lhsT=ones_row[:1, :128],
                                         rhs=alphas[:1, :1], start=True,
                                         stop=True)
                        a.copy(out=bc[:, :1], in_=psb2[:, :1])
                        a_ = bc[:, 0:1]
                        STT_V(out=b_r, in0=p_r, scalar=a_, in1=b_r,
                              op0=OP.mult, op1=OP.add)
                        STT_V(out=b_i, in0=p_i, scalar=a_, in1=b_i,
                              op0=OP.mult, op1=OP.add)
                        continue

                    # ---- per-sample scalars: pq (DVE mult+reduce), qq (ACT
                    # square+accum), and rr on iter 0
                    TT(out=dump, in0=q_r, in1=p_r, op=OP.mult)
                    v.reduce_sum(out=partials[:, 0:1], in_=dump,
                                 axis=mybir.AxisListType.X)
                    TT(out=tC, in0=q_i, in1=p_i, op=OP.mult)
                    v.reduce_sum(out=partials[:, 1:2], in_=tC,
                                 axis=mybir.AxisListType.X)
                    a.activation(out=tA, in_=q_r, func=ACTF.Square,
                                 accum_out=partials[:, 2:3])
                    a.activation(out=tB, in_=q_i, func=ACTF.Square,
                                 accum_out=partials[:, 3:4])
                    k = 4
                    if it == 0:
                        a.activation(out=tD, in_=r_r, func=ACTF.Square,
                                     accum_out=partials[:, 4:5])
                        a.activation(out=tE, in_=r_i, func=ACTF.Square,
                                     accum_out=partials[:, 5:6])
                        k = 6
                    ps1 = psp.tile([1, 16], f32, tag="mm")
                    nc.tensor.matmul(ps1[:1, :k], lhsT=ones_col[:, :1],
                                     rhs=partials[:, :k], start=True, stop=True)
                    a.copy(out=redsums[:1, :k], in_=ps1[:1, :k])
                    # pq = c0+c1, qq = c2+c3 (, rr = c4+c5)
                    TT(out=scl[:1, 0:1], in0=redsums[:1, 0:1],
                       in1=redsums[:1, 1:2], op=OP.add)
                    TT(out=scl[:1, 1:2], in0=redsums[:1, 2:3],
                       in1=redsums[:1, 3:4], op=OP.add)
                    if it == 0:
                        TT(out=rr_t[:1, :1], in0=redsums[:1, 4:5],
                           in1=redsums[:1, 5:6], op=OP.add)
                    v.reciprocal(out=scl[:1, 2:3], in_=scl[:1, 0:1])
                    TT(out=alphas[:1, 0:1], in0=rr_t[:1, :1],
                       in1=scl[:1, 2:3], op=OP.mult)          # alpha = rr/pq
                    TT(out=scl[:1, 3:4], in0=alphas[:1, 0:1],
                       in1=alphas[:1, 0:1], op=OP.mult)       # alpha^2
                    TT(out=scl[:1, 4:5], in0=scl[:1, 3:4],
                       in1=scl[:1, 1:2], op=OP.mult)          # alpha^2 qq
                    TT(out=rrn_t[:1, :1], in0=scl[:1, 4:5],
                       in1=rr_t[:1, :1], op=OP.subtract)      # rr_new
                    v.reciprocal(out=scl[:1, 5:6], in_=rr_t[:1, :1])
                    TT(out=alphas[:1, 2:3], in0=rrn_t[:1, :1],
                       in1=scl[:1, 5:6], op=OP.mult)          # beta
                    v.tensor_scalar_mul(out=alphas[:1, 1:2],
                                        in0=alphas[:1, 0:1], scalar1=-1.0)
                    a.copy(out=rr_t[:1, :1], in_=rrn_t[:1, :1])
                    psb2 = psp.tile([128, 16], f32, tag="mm")
                    nc.tensor.matmul(psb2[:, :3], lhsT=ones_row[:1, :128],
                                     rhs=alphas[:1, :3], start=True, stop=True)
                    a.copy(out=bc[:, :3], in_=psb2[:, :3])
                    a_ = bc[:, 0:1]
                    na = bc[:, 1:2]
                    bet = bc[:, 2:3]
                    # r -= alpha q ; p' = r + beta p ; b += alpha p (reads old
                    # p, emitted last - it doesn't gate the next iteration).
                    STT_V(out=r_r, in0=q_r, scalar=na, in1=r_r,
                          op0=OP.mult, op1=OP.add)
                    STT_V(out=pn_r, in0=p_r, scalar=bet, in1=r_r,
                          op0=OP.mult, op1=OP.add)
                    a.copy(out=pb_r, in_=pn_r)
                    STT_V(out=r_i, in0=q_i, scalar=na, in1=r_i,
                          op0=OP.mult, op1=OP.add)
                    STT_V(out=pn_i, in0=p_i, scalar=bet, in1=r_i,
                          op0=OP.mult, op1=OP.add)
                    a.copy(out=pb_i, in_=pn_i)
                    STT_V(out=b_r, in0=p_r, scalar=a_, in1=b_r,
                          op0=OP.mult, op1=OP.add)
                    STT_V(out=b_i, in0=p_i, scalar=a_, in1=b_i,
                          op0=OP.mult, op1=OP.add)

            store_folded(b_r, out_d[0])
            store_folded(b_i, out_d[1])

    nc.compile()
    return nc


def _host_inputs(Hc, Wc, packing, RTL, NB):
    bf = np.float16
    Fc = _centered_dft(Hc)
    f_r = np.ascontiguousarray(Fc.real).astype(np.float32)
    f_i = np.ascontiguousarray(Fc.imag).astype(np.float32)
    f_ni = (-f_i).astype(np.float32)
    shared = {"f_r": f_r.astype(bf), "f_i": f_i.astype(bf), "f_ni": f_ni.astype(bf)}
    if packing:
        t0 = 128 * (NB - 1)
        fr2 = f_r[t0:Hc, :]
        fi2 = f_i[t0:Hc, :]
        fni2 = f_ni[t0:Hc, :]
        fpk = np.concatenate(
            [
                np.concatenate([fr2, fni2], axis=0),
                np.concatenate([fi2, fr2], axis=0),
                np.concatenate([fr2, fi2], axis=0),
                np.concatenate([fni2, fr2], axis=0),
            ],
            axis=1,
        )
        shared["f_pk"] = np.ascontiguousarray(fpk).astype(bf)
    return shared


def prepare(us_image, reconstruction, mask, csm_r, csm_i, mu, reps=1):
    """Build (cached) the Bass module and per-core input maps."""
    bf = np.float16
    Bc, _, Hc, Wc = us_image.shape
    Cc = csm_r.shape[1]
    n_cores = Bc
    iters = CG_ITER

    BL = _blocks(Hc)
    NB = len(BL)
    RTL = BL[-1][1] if BL[-1][1] < 128 else 0
    packing = RTL > 0 and 2 * RTL <= 128

    key = (Hc, Wc, Cc, iters, n_cores, reps)
    if key not in _nc_cache:
        _nc_cache[key] = _build(Hc, Wc, Cc, iters, n_cores, reps=reps)
    nc = _nc_cache[key]

    shared = _host_inputs(Hc, Wc, packing, RTL, NB)

    in_maps = []
    for b in range(n_cores):
        m = {
            "us_image": np.ascontiguousarray(us_image[b], dtype=np.float32),
            "reconstruction": np.ascontiguousarray(
                reconstruction[b], dtype=np.float32
            ),
            "mask": np.ascontiguousarray(mask[b, 0]).astype(bf),
            "csm_r": np.ascontiguousarray(csm_r[b]).astype(bf),
            "csm_i": np.ascontiguousarray(csm_i[b]).astype(bf),
            "mu": np.ascontiguousarray(mu, dtype=np.float32),
        }
        m.update(shared)
        if packing:
            t0 = 128 * (NB - 1)
            m2 = np.ascontiguousarray(mask[b, 0, t0:Hc, :])
            m["mask_pk"] = np.concatenate([m2, m2], axis=0).astype(bf)
        in_maps.append(m)
    return in_maps, nc, n_cores


def kernel(us_image, reconstruction, mask, csm_r, csm_i, mu):
    global LAST_RESULT
    from concourse.bass_utils import run_bass_kernel_spmd

    in_maps, nc, n_cores = prepare(us_image, reconstruction, mask, csm_r, csm_i, mu)
    res = run_bass_kernel_spmd(nc, in_maps, core_ids=list(range(n_cores)))
    LAST_RESULT = res
    out = np.stack([res.results[b]["out"] for b in range(n_cores)], axis=0)
    return out.astype(np.float32)


# revision 23
# speedup vs baseline: 1.1477x; 1.0506x over previous
"""MRI data-consistency CG solver on 8 Trainium2 NeuronCores.

Sharding: pure data-parallel, 1 batch sample per core. The CG alpha/beta
scalars are computed per-sample (deviation from the reference's global
batch sums is ~3e-4 relative, far below tolerance), so cores run fully
independently - no collectives.

Per coil, the centered 2D FFT / IFFT are chained PE matmuls with the
centered DFT matrix Fc (Fc = S F S is symmetric, so
  stage1 = X^T Fc   (data as lhsT; output transposed)
  stage2 = stage1^T Fc = Fc X Fc  (natural orientation again)
-> no explicit transposes anywhere).

Data-path dtype: ALL matmul operands and csm-coupled elementwise ops
are fp16 (walrus rejects f32r x 2-byte mixes; fp16 runs at the same
1 cyc/row + DVE 2x_1p speed as bf16 with 4 more mantissa bits - bf16
measured 2.1e-2 final rel err vs fp16's 2.4e-3). 2-byte LDWEIGHTS
hides under the 320-col matmuls; PSUM accumulates fp32 and the CG
state (r/p/q/b) stays fp32.

All 32 csm half-tiles are SBUF-resident in fp16 (no per-iteration
re-streaming). CG scalars: pq = DVE mult+reduce, qq/rr = ACT
Square+accum_out (tensor_tensor_reduce crashes the device at runtime -
do not use it). The last CG iteration only needs
alpha = rr / p^H A p, and p^H A p = sum_c ||M F C_c p||^2 + mu ||p||^2,
so stages 3/4 + the coil combine are skipped there; the masked-kspace
norms accumulate via per-coil ACT squares.

The m2-tail pr/pi matmul pair writes ONE psum tile (pi directly on
partitions 64:128 via the PE column group), so every packed
evacuation is two partition-aligned engine copies - no SBUF->SBUF
partition-shift DMAs (eliminating them was worth ~170us; moving
qaccum multiplies to Pool was a ~310us regression - Pool's 0.42
efficiency paces the per-coil q chain - keep them on DVE).

Field layout: each 320x320 field lives in one SBUF tile [128, 3*320]
("folded"): block b (cols [b*320,(b+1)*320)) holds rows [b*128, ...) of
the matrix. Block 2 only uses partitions 0..63 in the standard layout
(junk kept at 0); the matmul-chain intermediates instead use a "packed"
layout where block 2 of the REAL tile holds [re rows 256:320 (p0:64);
im rows 256:320 (p64:128)], which lets the two 64-row contraction tails
merge into one full 128-row matmul (k2-packing).

CG scalar algebra: alpha is real (p^H A p real) and r^H q == p^H q by
A-conjugacy, so per iteration only two sums are needed:
  pq = sum(q_r p_r + q_i p_i),  qq = sum(|q|^2)
  alpha = rr/pq;  rr_new = alpha^2 qq - rr;  beta = rr_new/rr
"""

import numpy as np

CG_ITER = 10

_nc_cache = {}
LAST_RESULT = None


def _blocks(n):
    out = []
    r0 = 0
    while r0 < n:
        sz = min(128, n - r0)
        out.append((r0, sz))
        r0 += sz
    return out


def _centered_dft(n):
    # Columns of Fc = centered orthonormal DFT applied to unit vectors:
    # y = fftshift(fft(ifftshift(x))) = Fc @ x. Fc is symmetric for even n.
    eye = np.eye(n)
    Fc = np.fft.fftshift(
        np.fft.fft(np.fft.ifftshift(eye, axes=0), axis=0, norm="ortho"), axes=0
    )
    return Fc


def _build(Hc, Wc, Cc, iters, n_cores, reps=1, dma_pack=True):
    import concourse.bacc as bacc
    import concourse.mybir as mybir
    import concourse.tile as tile

    f32 = mybir.dt.float32
    f32r = mybir.dt.float32r
    bf16 = mybir.dt.float16  # "bf16" name kept; fp16 = same speed, 4 more mantissa bits
    OP = mybir.AluOpType
    ACTF = mybir.ActivationFunctionType

    nc = bacc.Bacc(trn_type="TRN2", num_devices=n_cores)

    BL = _blocks(Hc)
    NB = len(BL)
    FW = NB * Wc
    # size of the partial tail block (0 if H divides evenly)
    RTL = BL[-1][1] if BL[-1][1] < 128 else 0
    packing = RTL > 0 and 2 * RTL <= 128
    import os as _os
    if _os.environ.get("NO_PACK"):
        packing = False

    us = nc.dram_tensor("us_image", [2, Hc, Wc], f32, kind="ExternalInput")
    rec = nc.dram_tensor("reconstruction", [2, Hc, Wc], f32, kind="ExternalInput")
    mask_d = nc.dram_tensor("mask", [Hc, Wc], bf16, kind="ExternalInput")
    csm_r_d = nc.dram_tensor("csm_r", [Cc, Hc, Wc], bf16, kind="ExternalInput")
    csm_i_d = nc.dram_tensor("csm_i", [Cc, Hc, Wc], bf16, kind="ExternalInput")
    mu_d = nc.dram_tensor("mu", [1], f32, kind="ExternalInput")
    fr_d = nc.dram_tensor("f_r", [Hc, Hc], bf16, kind="ExternalInput")
    fi_d = nc.dram_tensor("f_i", [Hc, Hc], bf16, kind="ExternalInput")
    fni_d = nc.dram_tensor("f_ni", [Hc, Hc], bf16, kind="ExternalInput")
    if packing:
        # packed k2 rhs tiles: [Ga[tail rows] on p0:R ; Gb[tail] on pR:2R]
        # slots: 0=[fr;fni] 1=[fi;fr] 2=[fr;fi] 3=[fni;fr]
        fpk_d = nc.dram_tensor("f_pk", [2 * RTL, 4 * Wc], bf16, kind="ExternalInput")
        mpk_d = nc.dram_tensor("mask_pk", [2 * RTL, Wc], bf16, kind="ExternalInput")
    out_d = nc.dram_tensor("out", [2, Hc, Wc], f32, kind="ExternalOutput")

    with tile.TileContext(nc) as tc:
        with (
            tc.tile_pool(name="consts", bufs=1) as consts,
            tc.tile_pool(name="state", bufs=1) as state,
            tc.tile_pool(name="work", bufs=1) as work,
            tc.tile_pool(name="small", bufs=1) as small,
            tc.tile_pool(name="psum", bufs=8, space="PSUM") as psp,
        ):
            zero_f32 = []   # [128, FW] f32 tiles to memset (junk must be 0)
            zero_bf16 = []  # bf16 tiles whose junk feeds reductions

            def T(pool, name, shape, dtype=f32, zero=None):
                tl = pool.tile(shape, dtype, tag=name)
                if zero is None:
                    zero = list(shape) == [128, FW] and dtype == f32
                if zero:
                    (zero_f32 if dtype == f32 else zero_bf16).append(tl)
                return tl

            fr = T(consts, "fr", [128, FW], bf16)
            fi = T(consts, "fi", [128, FW], bf16)
            fni = T(consts, "fni", [128, FW], bf16)
            maskf = T(consts, "maskf", [128, FW], bf16)
            if packing:
                fpk = T(consts, "fpk", [2 * RTL, 4 * Wc], bf16)
                mpk = T(consts, "mpk", [2 * RTL, Wc], bf16)
            ones_col = T(consts, "ones_col", [128, 1])
            ones_row = T(consts, "ones_row", [1, 128])
            mu_b = T(consts, "mu_b", [128, 1])
            mu_sb = T(consts, "mu_sb", [1, 1])

            pp = [
                [T(state, "pA_r", [128, FW]), T(state, "pA_i", [128, FW])],
                [T(state, "pB_r", [128, FW]), T(state, "pB_i", [128, FW])],
            ]
            r_r = T(state, "r_r", [128, FW])
            r_i = T(state, "r_i", [128, FW])
            b_r = T(state, "b_r", [128, FW])
            b_i = T(state, "b_i", [128, FW])
            q_r = T(state, "q_r", [128, FW])
            q_i = T(state, "q_i", [128, FW])
            # bf16 shadow of p for the csm products
            pb_r = T(state, "pb_r", [128, FW], bf16, zero=True)
            pb_i = T(state, "pb_i", [128, FW], bf16, zero=True)

            cp = [[T(work, f"cp_{x}{j}", [128, FW], bf16) for x in "ri"] for j in (0, 1)]
            s1 = [[T(work, f"s1_{x}{j}", [128, FW], bf16) for x in "ri"] for j in (0, 1)]
            # km feeds full-tile TTR reductions on the last iteration
            km = [[T(work, f"km_{x}{j}", [128, FW], bf16, zero=True) for x in "ri"]
                  for j in (0, 1)]
            s3 = [[T(work, f"s3_{x}{j}", [128, FW], bf16) for x in "ri"] for j in (0, 1)]
            # zz feeds the fp32 q accumulation: junk must be 0
            zz = [[T(work, f"zz_{x}{j}", [128, FW], bf16, zero=True) for x in "ri"]
                  for j in (0, 1)]
            # all csm tiles SBUF-resident in bf16
            cs = [
                [T(work, f"cs_{x}{j}", [128, FW], bf16, zero=True) for x in "ri"]
                for j in range(Cc)
            ]
            pj1 = T(work, "pj1", [128, FW], bf16)
            pj2 = T(work, "pj2", [128, FW], bf16)
            pj3 = T(work, "pj3", [128, FW], bf16)
            pj4 = T(work, "pj4", [128, FW], bf16)
            tA = T(work, "tA", [128, FW], bf16)
            tB = T(work, "tB", [128, FW], bf16)
            tC = T(work, "tC", [128, FW], bf16)
            tD = T(work, "tD", [128, FW], bf16)
            tE = T(work, "tE", [128, FW], bf16)
            tF = T(work, "tF", [128, FW], bf16)
            dump = T(work, "dump", [128, FW], bf16)

            # cols: 0/1 pq parts, 2/3 qq parts, 4/5 rr parts,
            # 6..6+2C last-iter |masked ksp|^2 per coil, then 2 p-norm cols
            NPART = 8 + 2 * Cc
            partials = T(small, "partials", [128, NPART])
            redsums = T(small, "redsums", [1, NPART])
            scl = T(small, "scl", [1, 8])
            alphas = T(small, "alphas", [1, 4])
            bc = T(small, "bc", [128, 4])
            rr_t = T(small, "rr", [1, 1])
            rrn_t = T(small, "rrn", [1, 1])

            v = nc.vector
            g = nc.gpsimd
            a = nc.scalar
            STT_V = v.scalar_tensor_tensor
            TT = v.tensor_tensor

            # ---- init: zero everything (keeps junk regions at 0)
            for tl in zero_f32 + zero_bf16:
                v.memset(tl, 0.0)
            v.memset(partials, 0.0)
            v.memset(ones_col, 1.0)
            v.memset(ones_row, 1.0)

            def load_folded(dst, src2d):
                nbf = Hc // 128
                full = nbf * 128
                if nbf:
                    nc.sync.dma_start(
                        out=dst[:, 0 : nbf * Wc].rearrange("p (b w) -> p b w", b=nbf),
                        in_=src2d[0:full, :].rearrange("(b p) w -> p b w", p=128),
                    )
                if full < Hc:
                    rem = Hc - full
                    nc.sync.dma_start(
                        out=dst[:rem, nbf * Wc : (nbf + 1) * Wc],
                        in_=src2d[full:Hc, :],
                    )

            def store_folded(src, dst2d):
                nbf = Hc // 128
                full = nbf * 128
                if nbf:
                    nc.sync.dma_start(
                        out=dst2d[0:full, :].rearrange("(b p) w -> p b w", p=128),
                        in_=src[:, 0 : nbf * Wc].rearrange("p (b w) -> p b w", b=nbf),
                    )
                if full < Hc:
                    rem = Hc - full
                    nc.sync.dma_start(
                        out=dst2d[full:Hc, :],
                        in_=src[:rem, nbf * Wc : (nbf + 1) * Wc],
                    )

            # order: init-chain dependencies first (mu, us/rec for r, csm0/1
            # for the first projections), then the DFT matrices needed by
            # stage 1, then the rest
            nc.sync.dma_start(out=mu_sb[:1, :1], in_=mu_d[None, :])
            # borrow pp[1] and b for the r-init staging (all junk pre-zeroed)
            load_folded(pp[1][0], us[0])
            load_folded(pp[1][1], us[1])
            load_folded(b_r, rec[0])
            load_folded(b_i, rec[1])

            psb = psp.tile([128, 16], f32, tag="mm")
            nc.tensor.matmul(
                psb[:, :1], lhsT=ones_row[:1, :128], rhs=mu_sb[:1, :1],
                start=True, stop=True,
            )
            a.copy(out=mu_b[:, :1], in_=psb[:, :1])

            def load_csm(ci_):
                load_folded(cs[ci_][0], csm_r_d[ci_])
                load_folded(cs[ci_][1], csm_i_d[ci_])

            for j in range(min(Cc, 2)):
                load_csm(j)
            load_folded(fr, fr_d[:])
            load_folded(fi, fi_d[:])
            load_folded(fni, fni_d[:])
            if packing:
                nc.sync.dma_start(out=fpk, in_=fpk_d[:])
                nc.sync.dma_start(out=mpk, in_=mpk_d[:])
            load_folded(maskf, mask_d[:])
            for j in range(2, Cc):
                load_csm(j)

            # G-sets: (gr, gi, gni, pk_pr, pk_pi)
            if packing:
                gF = (fr, fi, fni, fpk[:, 0:Wc], fpk[:, Wc : 2 * Wc])
                gB = (fr, fni, fi, fpk[:, 2 * Wc : 3 * Wc], fpk[:, 3 * Wc : 4 * Wc])
            else:
                gF = (fr, fi, fni, None, None)
                gB = (fr, fni, fi, None, None)

            def mm_group2_head(out_a, wa, ra, out_b, wb, rb, has_tail):
                """Interleaved head matmuls of two accumulation groups.

                If has_tail, groups are left open for a deferred k2 pair.
                """
                n = len(wa)
                assert len(wb) == n
                for j in range(n):
                    stop = (not has_tail) and j == n - 1
                    nc.tensor.matmul(
                        out_a, lhsT=wa[j], rhs=ra[j], start=(j == 0), stop=stop,
                    )
                    nc.tensor.matmul(
                        out_b, lhsT=wb[j], rhs=rb[j], start=(j == 0), stop=stop,
                    )

            def mm_stage(xr, xi, gset, packed_in, packed_out, consume):
                """out = (xr + i xi)^T @ (gr + i gi); gni = -gi precomputed.

                packed_in: xr block NB-1 holds [re_tail; im_tail] (k2-packing)
                packed_out: m-tail block's imag half staged for partition shift
                """
                gr, gi, gni, pk_pr, pk_pi = gset

                def wslices(tl, m0, msz):
                    # weight slices per k-block: list of (ap, rhs) pairs
                    return [
                        tl[:ksz, k * Wc + m0 : k * Wc + m0 + msz]
                        for k, (k0, ksz) in enumerate(BL)
                    ]

                pending = []

                def flush_one():
                    # emit the deferred k2 pair of the oldest open m-block,
                    # close its groups, and evacuate
                    m, msz, pr, pi, tail = pending.pop(0)
                    if tail is not None:
                        (wk2, rk2_a, rk2_b) = tail
                        out_b = pi[:msz, :] if pi is not None else pr[64 : 64 + msz, :]
                        nc.tensor.matmul(
                            pr[:msz, :], lhsT=wk2, rhs=rk2_a,
                            start=False, stop=True,
                        )
                        nc.tensor.matmul(
                            out_b, lhsT=wk2, rhs=rk2_b,
                            start=False, stop=True,
                        )
                    consume(m, msz, pr, pi)

                for m in range(NB):
                    m0, msz = BL[m]
                    wr = wslices(xr, m0, msz)
                    wi = wslices(xi, m0, msz)
                    tail = None
                    if packed_in and packing:
                        # last k-block: single 128-row packed MM (deferred -
                        # its input is written by the previous stage's
                        # partition-shift, so give it pipeline slack)
                        w_pr = wr[:-1] + wi[:-1]
                        r_pr = [
                            gr[:128, k * Wc : (k + 1) * Wc] for k in range(NB - 1)
                        ] + [
                            gni[:128, k * Wc : (k + 1) * Wc] for k in range(NB - 1)
                        ]
                        w_pi = wr[:-1] + wi[:-1]
                        r_pi = [
                            gi[:128, k * Wc : (k + 1) * Wc] for k in range(NB - 1)
                        ] + [
                            gr[:128, k * Wc : (k + 1) * Wc] for k in range(NB - 1)
                        ]
                        tail = (
                            xr[: 2 * RTL, (NB - 1) * Wc + m0 : (NB - 1) * Wc + m0 + msz],
                            pk_pr,
                            pk_pi,
                        )
                    else:
                        w_pr = wr + wi
                        r_pr = [
                            gr[:ksz, k * Wc : (k + 1) * Wc]
                            for k, (k0, ksz) in enumerate(BL)
                        ] + [
                            gni[:ksz, k * Wc : (k + 1) * Wc]
                            for k, (k0, ksz) in enumerate(BL)
                        ]
                        w_pi = wr + wi
                        r_pi = [
                            gi[:ksz, k * Wc : (k + 1) * Wc]
                            for k, (k0, ksz) in enumerate(BL)
                        ] + [
                            gr[:ksz, k * Wc : (k + 1) * Wc]
                            for k, (k0, ksz) in enumerate(BL)
                        ]

                    pr = psp.tile([128, Wc], f32, tag="mm")
                    if packed_out and packing and m == NB - 1:
                        # pack the m-tail pair into ONE psum tile: pr rows on
                        # partitions 0:msz, pi rows on 64:64+msz (the PE
                        # writes the upper column group directly - no
                        # partition-shift DMA needed at evacuation)
                        pi = None
                        out_a = pr[:msz, :]
                        out_b = pr[64 : 64 + msz, :]
                    else:
                        pi = psp.tile([128, Wc], f32, tag="mm")
                        out_a = pr[:msz, :]
                        out_b = pi[:msz, :]
                    mm_group2_head(
                        out_a, w_pr, r_pr, out_b, w_pi, r_pi,
                        has_tail=tail is not None,
                    )
                    pending.append((m, msz, pr, pi, tail))
                    import os as _os
                    if len(pending) > (0 if _os.environ.get("NO_DEFER") else 1):
                        flush_one()
                while pending:
                    flush_one()

            def evac_copy(dst_r, dst_i, packed_out, sidx=0):
                def f(m, msz, pr, pi):
                    c0 = m * Wc
                    if pi is None:
                        # packed m-tail pair in one psum tile: both halves
                        # evacuate with partition-aligned engine copies
                        a.copy(out=dst_r[:msz, c0 : c0 + Wc], in_=pr[:msz, :])
                        a.copy(
                            out=dst_r[64 : 64 + msz, c0 : c0 + Wc],
                            in_=pr[64 : 64 + msz, :],
                        )
                        return
                    a.copy(out=dst_r[:msz, c0 : c0 + Wc], in_=pr[:msz, :])
                    a.copy(out=dst_i[:msz, c0 : c0 + Wc], in_=pi[:msz, :])
                return f

            def evac_mask(dst_r, dst_i, sidx=1):
                # must be DVE: only ACT/DVE can access PSUM, and ACT cannot
                # do tensor*tensor; pair-interleaving hides the latency
                def f(m, msz, pr, pi):
                    c0 = m * Wc
                    if pi is None:
                        v.tensor_tensor(
                            out=dst_r[:msz, c0 : c0 + Wc], in0=pr[:msz, :],
                            in1=mpk[:msz, :], op=OP.mult,
                        )
                        v.tensor_tensor(
                            out=dst_r[64 : 64 + msz, c0 : c0 + Wc],
                            in0=pr[64 : 64 + msz, :],
                            in1=mpk[64 : 64 + msz, :], op=OP.mult,
                        )
                        return
                    mk = maskf[:msz, c0 : c0 + Wc]
                    v.tensor_tensor(
                        out=dst_r[:msz, c0 : c0 + Wc], in0=pr[:msz, :], in1=mk,
                        op=OP.mult,
                    )
                    v.tensor_tensor(
                        out=dst_i[:msz, c0 : c0 + Wc], in0=pi[:msz, :], in1=mk,
                        op=OP.mult,
                    )
                return f

            def proj(c):
                """Coil projection cp = p * csm (all-bf16).

                DVE carries 4 ops (2x_1p), Pool 2. Emitted ahead of the
                consuming stages so the elementwise engines compute it while
                PE runs earlier coils' stages.
                """
                slot = c % 2
                csr, csi = cs[c]
                # pj1 on Pool runs concurrently with DVE's pj2/pj3/pj4 -
                # the combine chain depth is ~2.4us instead of Pool-paced
                # 4.3us (proj gates stage 1 of every coil)
                g.tensor_tensor(out=pj1, in0=pb_r, in1=csr, op=OP.mult)
                v.tensor_tensor(out=pj2, in0=pb_i, in1=csi, op=OP.mult)
                v.tensor_tensor(out=pj3, in0=pb_r, in1=csi, op=OP.mult)
                v.tensor_tensor(out=pj4, in0=pb_i, in1=csr, op=OP.mult)
                v.tensor_tensor(out=cp[slot][0], in0=pj1, in1=pj2, op=OP.subtract)
                v.tensor_tensor(out=cp[slot][1], in0=pj3, in1=pj4, op=OP.add)
                if packing and dma_pack:
                    # pack cp: copy im tail rows into cp_r block NB-1 p64:128
                    # (partition-shifting SBUF->SBUF DMA)
                    nc.sync.dma_start(
                        out=cp[slot][0][64 : 64 + RTL, (NB - 1) * Wc : NB * Wc],
                        in_=cp[slot][1][0:RTL, (NB - 1) * Wc : NB * Wc],
                    )

            def stage(stg, c):
                slot = c % 2
                stage1_packed_in = bool(packing and dma_pack)
                if stg == 0:
                    mm_stage(cp[slot][0], cp[slot][1], gF, stage1_packed_in,
                             True, evac_copy(s1[slot][0], s1[slot][1], True, 0))
                elif stg == 1:
                    mm_stage(s1[slot][0], s1[slot][1], gF, True, True,
                             evac_mask(km[slot][0], km[slot][1]))
                elif stg == 2:
                    mm_stage(km[slot][0], km[slot][1], gB, True, True,
                             evac_copy(s3[slot][0], s3[slot][1], True, 2))
                else:
                    mm_stage(s3[slot][0], s3[slot][1], gB, True, False,
                             evac_copy(zz[slot][0], zz[slot][1], False))

            def qaccum(c):
                # q += z * conj(csm); bf16 products/pair-sums on DVE, the
                # fp32 accumulate split DVE (re) / Pool (im)
                slot = c % 2
                csr, csi = cs[c]
                zr, zi = zz[slot]
                v.tensor_tensor(out=tA, in0=zr, in1=csr, op=OP.mult)
                v.tensor_tensor(out=tB, in0=zi, in1=csi, op=OP.mult)
                v.tensor_tensor(out=tC, in0=tA, in1=tB, op=OP.add)
                v.tensor_tensor(out=q_r, in0=q_r, in1=tC, op=OP.add)
                v.tensor_tensor(out=tD, in0=zi, in1=csr, op=OP.mult)
                v.tensor_tensor(out=tE, in0=zr, in1=csi, op=OP.mult)
                v.tensor_tensor(out=tF, in0=tD, in1=tE, op=OP.subtract)
                if c == Cc - 1:
                    # the last coil's q_i accumulate gates the iteration-end
                    # reductions - keep it off the slow Pool engine
                    v.tensor_tensor(out=q_i, in0=q_i, in1=tF, op=OP.add)
                else:
                    g.tensor_tensor(out=q_i, in0=q_i, in1=tF, op=OP.add)

            # last-iteration |masked kspace|^2: ACT Square+accum, one
            # partials column per (coil, component)
            def ksq_accum(c):
                slot = c % 2
                kr, ki = km[slot]
                col = 6 + 2 * c
                a.activation(out=tA, in_=kr, func=ACTF.Square,
                             accum_out=partials[:, col : col + 1])
                a.activation(out=tB, in_=ki, func=ACTF.Square,
                             accum_out=partials[:, col + 1 : col + 2])

            for rep in range(reps):
                # ---- (re)init: r = us + mu*rec; p = r; b = 0
                if rep > 0:
                    load_folded(pp[1][0], us[0])
                    load_folded(pp[1][1], us[1])
                    load_folded(b_r, rec[0])
                    load_folded(b_i, rec[1])
                STT_V(out=r_r, in0=b_r, scalar=mu_b[:, :1], in1=pp[1][0],
                      op0=OP.mult, op1=OP.add)
                STT_V(out=r_i, in0=b_i, scalar=mu_b[:, :1], in1=pp[1][1],
                      op0=OP.mult, op1=OP.add)
                a.copy(out=pp[0][0], in_=r_r)
                a.copy(out=pp[0][1], in_=r_i)
                a.copy(out=pb_r, in_=r_r)
                a.copy(out=pb_i, in_=r_i)
                v.memset(b_r, 0.0)
                v.memset(b_i, 0.0)

                for it in range(iters):
                    p_r, p_i = pp[it % 2]
                    pn_r, pn_i = pp[(it + 1) % 2]
                    last = it + 1 == iters and rep + 1 == reps
                    # q = mu * p (coils accumulate on top); ACT scale-copy
                    if not last:
                        a.activation(out=q_r, in_=p_r, func=ACTF.Copy,
                                     scale=mu_b[:, :1])
                        a.activation(out=q_i, in_=p_i, func=ACTF.Copy,
                                     scale=mu_b[:, :1])
                    proj(0)
                    if Cc > 1:
                        proj(1)
                    n_stg = 2 if last else 4
                    # pairwise stage interleave: s1(c) s1(c+1) s2(c) s2(c+1)
                    # ... so every inter-stage evacuation hides behind the
                    # sibling coil's matmuls
                    for c in range(0, Cc, 2):
                        cset = [c] + ([c + 1] if c + 1 < Cc else [])
                        for stg in range(n_stg):
                            for cc in cset:
                                stage(stg, cc)
                                if stg == 3:
                                    qaccum(cc)
                                if last and stg == 1:
                                    ksq_accum(cc)
                            if stg == 0:
                                # next pair's projections (cp slots freed by
                                # this pair's stage-1 reads)
                                for cc in cset:
                                    if cc + 2 < Cc:
                                        proj(cc + 2)
                    if last:
                        # pq = sum_c ||M F C_c p||^2 + mu ||p||^2
                        pn0 = 6 + 2 * Cc
                        a.activation(out=dump, in_=p_r, func=ACTF.Square,
                                     accum_out=partials[:, pn0 : pn0 + 1])
                        a.activation(out=dump, in_=p_i, func=ACTF.Square,
                                     accum_out=partials[:, pn0 + 1 : pn0 + 2])
                        k = pn0 + 2
                        ps1 = psp.tile([1, NPART], f32, tag="mm")
                        nc.tensor.matmul(ps1[:1, :k], lhsT=ones_col[:, :1],
                                         rhs=partials[:, :k], start=True,
                                         stop=True)
                        a.copy(out=redsums[:1, :k], in_=ps1[:1, :k])
                        # sum the 2C per-coil kspace columns in one reduce
                        v.reduce_sum(out=scl[:1, 0:1],
                                     in_=redsums[:1, 6 : 6 + 2 * Cc],
                                     axis=mybir.AxisListType.X)
                        TT(out=scl[:1, 1:2], in0=redsums[:1, pn0 : pn0 + 1],
                           in1=redsums[:1, pn0 + 1 : pn0 + 2], op=OP.add)
                        # pq = ksq + mu * pnorm
                        STT_V(out=scl[:1, 2:3], in0=scl[:1, 1:2],
                              scalar=mu_sb[:1, :1], in1=scl[:1, 0:1],
                              op0=OP.mult, op1=OP.add)
                        v.reciprocal(out=scl[:1, 3:4], in_=scl[:1, 2:3])
                        TT(out=alphas[:1, 0:1], in0=rr_t[:1, :1],
                           in1=scl[:1, 3:4], op=OP.mult)      # alpha = rr/pq
                        psb2 = psp.tile([128, 16], f32, tag="mm")
                        nc.tensor.matmul(psb2[:, :1], lhsT=ones_row[:1, :128],
                                         rhs=alphas[:1, :1], start=True,
                                         stop=True)
                        a.copy(out=bc[:, :1], in_=psb2[:, :1])
                        a_ = bc[:, 0:1]
                        STT_V(out=b_r, in0=p_r, scalar=a_, in1=b_r,
                              op0=OP.mult, op1=OP.add)
                        STT_V(out=b_i, in0=p_i, scalar=a_, in1=b_i,
                              op0=OP.mult, op1=OP.add)
                        continue

                    # ---- per-sample scalars: pq (DVE mult+reduce), qq (ACT
                    # square+accum), and rr on iter 0. ACT squares emitted
                    # first so they run concurrently with the DVE pq chain.
                    a.activation(out=tA, in_=q_r, func=ACTF.Square,
                                 accum_out=partials[:, 2:3])
                    a.activation(out=tB, in_=q_i, func=ACTF.Square,
                                 accum_out=partials[:, 3:4])
                    TT(out=dump, in0=q_r, in1=p_r, op=OP.mult)
                    v.reduce_sum(out=partials[:, 0:1], in_=dump,
                                 axis=mybir.AxisListType.X)
                    TT(out=tC, in0=q_i, in1=p_i, op=OP.mult)
                    v.reduce_sum(out=partials[:, 1:2], in_=tC,
                                 axis=mybir.AxisListType.X)
                    k = 4
                    if it == 0:
                        a.activation(out=tD, in_=r_r, func=ACTF.Square,
                                     accum_out=partials[:, 4:5])
                        a.activation(out=tE, in_=r_i, func=ACTF.Square,
                                     accum_out=partials[:, 5:6])
                        k = 6
                    ps1 = psp.tile([1, 16], f32, tag="mm")
                    nc.tensor.matmul(ps1[:1, :k], lhsT=ones_col[:, :1],
                                     rhs=partials[:, :k], start=True, stop=True)
                    a.copy(out=redsums[:1, :k], in_=ps1[:1, :k])
                    # pq = c0+c1, qq = c2+c3 (, rr = c4+c5)
                    TT(out=scl[:1, 0:1], in0=redsums[:1, 0:1],
                       in1=redsums[:1, 1:2], op=OP.add)
                    TT(out=scl[:1, 1:2], in0=redsums[:1, 2:3],
                       in1=redsums[:1, 3:4], op=OP.add)
                    if it == 0:
                        TT(out=rr_t[:1, :1], in0=redsums[:1, 4:5],
                           in1=redsums[:1, 5:6], op=OP.add)
                    v.reciprocal(out=scl[:1, 2:3], in_=scl[:1, 0:1])
                    TT(out=alphas[:1, 0:1], in0=rr_t[:1, :1],
                       in1=scl[:1, 2:3], op=OP.mult)          # alpha = rr/pq
                    TT(out=scl[:1, 3:4], in0=alphas[:1, 0:1],
                       in1=alphas[:1, 0:1], op=OP.mult)       # alpha^2
                    TT(out=scl[:1, 4:5], in0=scl[:1, 3:4],
                       in1=scl[:1, 1:2], op=OP.mult)          # alpha^2 qq
                    TT(out=rrn_t[:1, :1], in0=scl[:1, 4:5],
                       in1=rr_t[:1, :1], op=OP.subtract)      # rr_new
                    v.reciprocal(out=scl[:1, 5:6], in_=rr_t[:1, :1])
                    TT(out=alphas[:1, 2:3], in0=rrn_t[:1, :1],
                       in1=scl[:1, 5:6], op=OP.mult)          # beta
                    v.tensor_scalar_mul(out=alphas[:1, 1:2],
                                        in0=alphas[:1, 0:1], scalar1=-1.0)
                    a.copy(out=rr_t[:1, :1], in_=rrn_t[:1, :1])
                    psb2 = psp.tile([128, 16], f32, tag="mm")
                    nc.tensor.matmul(psb2[:, :3], lhsT=ones_row[:1, :128],
                                     rhs=alphas[:1, :3], start=True, stop=True)
                    a.copy(out=bc[:, :3], in_=psb2[:, :3])
                    a_ = bc[:, 0:1]
                    na = bc[:, 1:2]
                    bet = bc[:, 2:3]
                    # r -= alpha q ; p' = r + beta p ; b += alpha p (reads old
                    # p, emitted last - it doesn't gate the next iteration).
                    STT_V(out=r_r, in0=q_r, scalar=na, in1=r_r,
                          op0=OP.mult, op1=OP.add)
                    STT_V(out=pn_r, in0=p_r, scalar=bet, in1=r_r,
                          op0=OP.mult, op1=OP.add)
                    a.copy(out=pb_r, in_=pn_r)
                    STT_V(out=r_i, in0=q_i, scalar=na, in1=r_i,
                          op0=OP.mult, op1=OP.add)
                    STT_V(out=pn_i, in0=p_i, scalar=bet, in1=r_i,
                          op0=OP.mult, op1=OP.add)
                    a.copy(out=pb_i, in_=pn_i)
                    STT_V(out=b_r, in0=p_r, scalar=a_, in1=b_r,
                          op0=OP.mult, op1=OP.add)
                    STT_V(out=b_i, in0=p_i, scalar=a_, in1=b_i,
                          op0=OP.mult, op1=OP.add)

            store_folded(b_r, out_d[0])
            store_folded(b_i, out_d[1])

    nc.compile()
    return nc


def _host_inputs(Hc, Wc, packing, RTL, NB):
    bf = np.float16
    Fc = _centered_dft(Hc)
    f_r = np.ascontiguousarray(Fc.real).astype(np.float32)
    f_i = np.ascontiguousarray(Fc.imag).astype(np.float32)
    f_ni = (-f_i).astype(np.float32)
    shared = {"f_r": f_r.astype(bf), "f_i": f_i.astype(bf), "f_ni": f_ni.astype(bf)}
    if packing:
        t0 = 128 * (NB - 1)
        fr2 = f_r[t0:Hc, :]
        fi2 = f_i[t0:Hc, :]
        fni2 = f_ni[t0:Hc, :]
        fpk = np.concatenate(
            [
                np.concatenate([fr2, fni2], axis=0),
                np.concatenate([fi2, fr2], axis=0),
                np.concatenate([fr2, fi2], axis=0),
                np.concatenate([fni2, fr2], axis=0),
            ],
            axis=1,
        )
        shared["f_pk"] = np.ascontiguousarray(fpk).astype(bf)
    return shared


def prepare(us_image, reconstruction, mask, csm_r, csm_i, mu, reps=1):
    """Build (cached) the Bass module and per-core input maps."""
    bf = np.float16
    Bc, _, Hc, Wc = us_image.shape
    Cc = csm_r.shape[1]
    n_cores = Bc
    iters = CG_ITER

    BL = _blocks(Hc)
    NB = len(BL)
    RTL = BL[-1][1] if BL[-1][1] < 128 else 0
    packing = RTL > 0 and 2 * RTL <= 128

    key = (Hc, Wc, Cc, iters, n_cores, reps)
    if key not in _nc_cache:
        _nc_cache[key] = _build(Hc, Wc, Cc, iters, n_cores, reps=reps)
    nc = _nc_cache[key]

    shared = _host_inputs(Hc, Wc, packing, RTL, NB)

    in_maps = []
    for b in range(n_cores):
        m = {
            "us_image": np.ascontiguousarray(us_image[b], dtype=np.float32),
            "reconstruction": np.ascontiguousarray(
                reconstruction[b], dtype=np.float32
            ),
            "mask": np.ascontiguousarray(mask[b, 0]).astype(bf),
            "csm_r": np.ascontiguousarray(csm_r[b]).astype(bf),
            "csm_i": np.ascontiguousarray(csm_i[b]).astype(bf),
            "mu": np.ascontiguousarray(mu, dtype=np.float32),
        }
        m.update(shared)
        if packing:
            t0 = 128 * (NB - 1)
            m2 = np.ascontiguousarray(mask[b, 0, t0:Hc, :])
            m["mask_pk"] = np.concatenate([m2, m2], axis=0).astype(bf)
        in_maps.append(m)
    return in_maps, nc, n_cores


def kernel(us_image, reconstruction, mask, csm_r, csm_i, mu):
    global LAST_RESULT
    from concourse.bass_utils import run_bass_kernel_spmd

    in_maps, nc, n_cores = prepare(us_image, reconstruction, mask, csm_r, csm_i, mu)
    res = run_bass_kernel_spmd(nc, in_maps, core_ids=list(range(n_cores)))
    LAST_RESULT = res
    out = np.stack([res.results[b]["out"] for b in range(n_cores)], axis=0)
    return out.astype(np.float32)


# revision 25
# speedup vs baseline: 1.1948x; 1.0410x over previous
"""MRI data-consistency CG solver on 8 Trainium2 NeuronCores.

Sharding: pure data-parallel, 1 batch sample per core. The CG alpha/beta
scalars are computed per-sample (deviation from the reference's global
batch sums is ~3e-4 relative, far below tolerance), so cores run fully
independently - no collectives.

Per coil, the centered 2D FFT / IFFT are chained PE matmuls with the
centered DFT matrix Fc (Fc = S F S is symmetric, so
  stage1 = X^T Fc   (data as lhsT; output transposed)
  stage2 = stage1^T Fc = Fc X Fc  (natural orientation again)
-> no explicit transposes anywhere).

Data-path dtype: ALL matmul operands and csm-coupled elementwise ops
are fp16 (walrus rejects f32r x 2-byte mixes; fp16 runs at the same
1 cyc/row + DVE 2x_1p speed as bf16 with 4 more mantissa bits - bf16
measured 2.1e-2 final rel err vs fp16's 2.4e-3). 2-byte LDWEIGHTS
hides under the 320-col matmuls; PSUM accumulates fp32 and the CG
state (r/p/q/b) stays fp32.

All 32 csm half-tiles are SBUF-resident in fp16 (no per-iteration
re-streaming). CG scalars: pq = DVE mult+reduce, qq/rr = ACT
Square+accum_out (tensor_tensor_reduce crashes the device at runtime -
do not use it). The last CG iteration only needs
alpha = rr / p^H A p, and p^H A p = sum_c ||M F C_c p||^2 + mu ||p||^2,
so stages 3/4 + the coil combine are skipped there; the masked-kspace
norms accumulate via per-coil ACT squares.

The m2-tail pr/pi matmul pair writes ONE psum tile (pi directly on
partitions 64:128 via the PE column group), so every packed
evacuation is two partition-aligned engine copies - no SBUF->SBUF
partition-shift DMAs (eliminating them was worth ~170us; moving
qaccum multiplies to Pool was a ~310us regression - Pool's 0.42
efficiency paces the per-coil q chain - keep them on DVE).

Field layout: each 320x320 field lives in one SBUF tile [128, 3*320]
("folded"): block b (cols [b*320,(b+1)*320)) holds rows [b*128, ...) of
the matrix. Block 2 only uses partitions 0..63 in the standard layout
(junk kept at 0); the matmul-chain intermediates instead use a "packed"
layout where block 2 of the REAL tile holds [re rows 256:320 (p0:64);
im rows 256:320 (p64:128)], which lets the two 64-row contraction tails
merge into one full 128-row matmul (k2-packing).

CG scalar algebra: alpha is real (p^H A p real) and r^H q == p^H q by
A-conjugacy, so per iteration only two sums are needed:
  pq = sum(q_r p_r + q_i p_i),  qq = sum(|q|^2)
  alpha = rr/pq;  rr_new = alpha^2 qq - rr;  beta = rr_new/rr
"""

import numpy as np

CG_ITER = 10

_nc_cache = {}
LAST_RESULT = None


def _blocks(n):
    out = []
    r0 = 0
    while r0 < n:
        sz = min(128, n - r0)
        out.append((r0, sz))
        r0 += sz
    return out


def _centered_dft(n):
    # Columns of Fc = centered orthonormal DFT applied to unit vectors:
    # y = fftshift(fft(ifftshift(x))) = Fc @ x. Fc is symmetric for even n.
    eye = np.eye(n)
    Fc = np.fft.fftshift(
        np.fft.fft(np.fft.ifftshift(eye, axes=0), axis=0, norm="ortho"), axes=0
    )
    return Fc


def _build(Hc, Wc, Cc, iters, n_cores, reps=1, dma_pack=True):
    import concourse.bacc as bacc
    import concourse.mybir as mybir
    import concourse.tile as tile

    f32 = mybir.dt.float32
    f32r = mybir.dt.float32r
    bf16 = mybir.dt.float16  # "bf16" name kept; fp16 = same speed, 4 more mantissa bits
    OP = mybir.AluOpType
    ACTF = mybir.ActivationFunctionType

    nc = bacc.Bacc(trn_type="TRN2", num_devices=n_cores)

    BL = _blocks(Hc)
    NB = len(BL)
    FW = NB * Wc
    # size of the partial tail block (0 if H divides evenly)
    RTL = BL[-1][1] if BL[-1][1] < 128 else 0
    packing = RTL > 0 and 2 * RTL <= 128
    import os as _os
    if _os.environ.get("NO_PACK"):
        packing = False

    us = nc.dram_tensor("us_image", [2, Hc, Wc], f32, kind="ExternalInput")
    rec = nc.dram_tensor("reconstruction", [2, Hc, Wc], f32, kind="ExternalInput")
    mask_d = nc.dram_tensor("mask", [Hc, Wc], bf16, kind="ExternalInput")
    csm_r_d = nc.dram_tensor("csm_r", [Cc, Hc, Wc], bf16, kind="ExternalInput")
    csm_i_d = nc.dram_tensor("csm_i", [Cc, Hc, Wc], bf16, kind="ExternalInput")
    mu_d = nc.dram_tensor("mu", [1], f32, kind="ExternalInput")
    fr_d = nc.dram_tensor("f_r", [Hc, Hc], bf16, kind="ExternalInput")
    fi_d = nc.dram_tensor("f_i", [Hc, Hc], bf16, kind="ExternalInput")
    fni_d = nc.dram_tensor("f_ni", [Hc, Hc], bf16, kind="ExternalInput")
    if packing:
        # packed k2 rhs tiles: [Ga[tail rows] on p0:R ; Gb[tail] on pR:2R]
        # slots: 0=[fr;fni] 1=[fi;fr] 2=[fr;fi] 3=[fni;fr]
        fpk_d = nc.dram_tensor("f_pk", [2 * RTL, 4 * Wc], bf16, kind="ExternalInput")
        mpk_d = nc.dram_tensor("mask_pk", [2 * RTL, Wc], bf16, kind="ExternalInput")
    out_d = nc.dram_tensor("out", [2, Hc, Wc], f32, kind="ExternalOutput")

    with tile.TileContext(nc) as tc:
        with (
            tc.tile_pool(name="consts", bufs=1) as consts,
            tc.tile_pool(name="state", bufs=1) as state,
            tc.tile_pool(name="work", bufs=1) as work,
            tc.tile_pool(name="small", bufs=1) as small,
            tc.tile_pool(name="psum", bufs=8, space="PSUM") as psp,
        ):
            zero_f32 = []   # [128, FW] f32 tiles to memset (junk must be 0)
            zero_bf16 = []  # bf16 tiles whose junk feeds reductions

            def T(pool, name, shape, dtype=f32, zero=None):
                tl = pool.tile(shape, dtype, tag=name)
                if zero is None:
                    zero = list(shape) == [128, FW] and dtype == f32
                if zero:
                    (zero_f32 if dtype == f32 else zero_bf16).append(tl)
                return tl

            fr = T(consts, "fr", [128, FW], bf16)
            fi = T(consts, "fi", [128, FW], bf16)
            fni = T(consts, "fni", [128, FW], bf16)
            maskf = T(consts, "maskf", [128, FW], bf16)
            if packing:
                fpk = T(consts, "fpk", [2 * RTL, 4 * Wc], bf16)
                mpk = T(consts, "mpk", [2 * RTL, Wc], bf16)
            ones_col = T(consts, "ones_col", [128, 1])
            ones_row = T(consts, "ones_row", [1, 128])
            mu_b = T(consts, "mu_b", [128, 1])
            mu_sb = T(consts, "mu_sb", [1, 1])

            pp = [
                [T(state, "pA_r", [128, FW]), T(state, "pA_i", [128, FW])],
                [T(state, "pB_r", [128, FW]), T(state, "pB_i", [128, FW])],
            ]
            r_r = T(state, "r_r", [128, FW])
            r_i = T(state, "r_i", [128, FW])
            b_r = T(state, "b_r", [128, FW])
            b_i = T(state, "b_i", [128, FW])
            q_r = T(state, "q_r", [128, FW])
            q_i = T(state, "q_i", [128, FW])
            # bf16 shadow of p for the csm products
            pb_r = T(state, "pb_r", [128, FW], bf16, zero=True)
            pb_i = T(state, "pb_i", [128, FW], bf16, zero=True)

            cp = [[T(work, f"cp_{x}{j}", [128, FW], bf16) for x in "ri"] for j in (0, 1)]
            s1 = [[T(work, f"s1_{x}{j}", [128, FW], bf16) for x in "ri"] for j in (0, 1)]
            # km feeds full-tile TTR reductions on the last iteration
            km = [[T(work, f"km_{x}{j}", [128, FW], bf16, zero=True) for x in "ri"]
                  for j in (0, 1)]
            s3 = [[T(work, f"s3_{x}{j}", [128, FW], bf16) for x in "ri"] for j in (0, 1)]
            # zz feeds the fp32 q accumulation: junk must be 0
            zz = [[T(work, f"zz_{x}{j}", [128, FW], bf16, zero=True) for x in "ri"]
                  for j in (0, 1)]
            # all csm tiles SBUF-resident in bf16
            cs = [
                [T(work, f"cs_{x}{j}", [128, FW], bf16, zero=True) for x in "ri"]
                for j in range(Cc)
            ]
            pj1 = T(work, "pj1", [128, FW], bf16)
            pj2 = T(work, "pj2", [128, FW], bf16)
            pj3 = T(work, "pj3", [128, FW], bf16)
            pj4 = T(work, "pj4", [128, FW], bf16)
            tA = T(work, "tA", [128, FW], bf16)
            tB = T(work, "tB", [128, FW], bf16)
            tC = T(work, "tC", [128, FW], bf16)
            tD = T(work, "tD", [128, FW], bf16)
            tE = T(work, "tE", [128, FW], bf16)
            tF = T(work, "tF", [128, FW], bf16)
            dump = T(work, "dump", [128, FW], bf16)

            # cols: 0/1 pq parts, 2/3 qq parts, 4/5 rr parts,
            # 6..6+2C last-iter |masked ksp|^2 per coil, then 2 p-norm cols
            NPART = 8 + 2 * Cc
            partials = T(small, "partials", [128, NPART])
            redsums = T(small, "redsums", [1, NPART])
            scl = T(small, "scl", [1, 8])
            alphas = T(small, "alphas", [1, 4])
            bc = T(small, "bc", [128, 4])
            rr_t = T(small, "rr", [1, 1])
            rrn_t = T(small, "rrn", [1, 1])

            v = nc.vector
            g = nc.gpsimd
            a = nc.scalar
            STT_V = v.scalar_tensor_tensor
            TT = v.tensor_tensor

            # ---- init: zero everything (keeps junk regions at 0)
            for tl in zero_f32 + zero_bf16:
                v.memset(tl, 0.0)
            v.memset(partials, 0.0)
            v.memset(ones_col, 1.0)
            v.memset(ones_row, 1.0)

            def load_folded(dst, src2d):
                nbf = Hc // 128
                full = nbf * 128
                if nbf:
                    nc.sync.dma_start(
                        out=dst[:, 0 : nbf * Wc].rearrange("p (b w) -> p b w", b=nbf),
                        in_=src2d[0:full, :].rearrange("(b p) w -> p b w", p=128),
                    )
                if full < Hc:
                    rem = Hc - full
                    nc.sync.dma_start(
                        out=dst[:rem, nbf * Wc : (nbf + 1) * Wc],
                        in_=src2d[full:Hc, :],
                    )

            def store_folded(src, dst2d):
                nbf = Hc // 128
                full = nbf * 128
                if nbf:
                    nc.sync.dma_start(
                        out=dst2d[0:full, :].rearrange("(b p) w -> p b w", p=128),
                        in_=src[:, 0 : nbf * Wc].rearrange("p (b w) -> p b w", b=nbf),
                    )
                if full < Hc:
                    rem = Hc - full
                    nc.sync.dma_start(
                        out=dst2d[full:Hc, :],
                        in_=src[:rem, nbf * Wc : (nbf + 1) * Wc],
                    )

            # order: init-chain dependencies first (mu, us/rec for r, csm0/1
            # for the first projections), then the DFT matrices needed by
            # stage 1, then the rest
            nc.sync.dma_start(out=mu_sb[:1, :1], in_=mu_d[None, :])
            # borrow pp[1] and b for the r-init staging (all junk pre-zeroed)
            load_folded(pp[1][0], us[0])
            load_folded(pp[1][1], us[1])
            load_folded(b_r, rec[0])
            load_folded(b_i, rec[1])

            psb = psp.tile([128, 16], f32, tag="mm")
            nc.tensor.matmul(
                psb[:, :1], lhsT=ones_row[:1, :128], rhs=mu_sb[:1, :1],
                start=True, stop=True,
            )
            a.copy(out=mu_b[:, :1], in_=psb[:, :1])

            def load_csm(ci_):
                load_folded(cs[ci_][0], csm_r_d[ci_])
                load_folded(cs[ci_][1], csm_i_d[ci_])

            for j in range(min(Cc, 2)):
                load_csm(j)
            load_folded(fr, fr_d[:])
            load_folded(fi, fi_d[:])
            load_folded(fni, fni_d[:])
            if packing:
                nc.sync.dma_start(out=fpk, in_=fpk_d[:])
                nc.sync.dma_start(out=mpk, in_=mpk_d[:])
            load_folded(maskf, mask_d[:])
            for j in range(2, Cc):
                load_csm(j)

            # G-sets: (gr, gi, gni, pk_pr, pk_pi)
            if packing:
                gF = (fr, fi, fni, fpk[:, 0:Wc], fpk[:, Wc : 2 * Wc])
                gB = (fr, fni, fi, fpk[:, 2 * Wc : 3 * Wc], fpk[:, 3 * Wc : 4 * Wc])
            else:
                gF = (fr, fi, fni, None, None)
                gB = (fr, fni, fi, None, None)

            def mm_group2_head(out_a, wa, ra, out_b, wb, rb, has_tail):
                """Interleaved head matmuls of two accumulation groups.

                If has_tail, groups are left open for a deferred k2 pair.
                """
                n = len(wa)
                assert len(wb) == n
                for j in range(n):
                    stop = (not has_tail) and j == n - 1
                    nc.tensor.matmul(
                        out_a, lhsT=wa[j], rhs=ra[j], start=(j == 0), stop=stop,
                    )
                    nc.tensor.matmul(
                        out_b, lhsT=wb[j], rhs=rb[j], start=(j == 0), stop=stop,
                    )

            def mm_stage(xr, xi, gset, packed_in, packed_out, consume):
                """out = (xr + i xi)^T @ (gr + i gi); gni = -gi precomputed.

                packed_in: xr block NB-1 holds [re_tail; im_tail] (k2-packing)
                packed_out: m-tail block's imag half staged for partition shift
                """
                gr, gi, gni, pk_pr, pk_pi = gset

                def wslices(tl, m0, msz):
                    # weight slices per k-block: list of (ap, rhs) pairs
                    return [
                        tl[:ksz, k * Wc + m0 : k * Wc + m0 + msz]
                        for k, (k0, ksz) in enumerate(BL)
                    ]

                pending = []

                def flush_one():
                    # emit the deferred k2 pair of the oldest open m-block,
                    # close its groups, and evacuate
                    m, msz, pr, pi, tail = pending.pop(0)
                    if tail is not None:
                        (wk2, rk2_a, rk2_b) = tail
                        out_b = pi[:msz, :] if pi is not None else pr[64 : 64 + msz, :]
                        nc.tensor.matmul(
                            pr[:msz, :], lhsT=wk2, rhs=rk2_a,
                            start=False, stop=True,
                        )
                        nc.tensor.matmul(
                            out_b, lhsT=wk2, rhs=rk2_b,
                            start=False, stop=True,
                        )
                    consume(m, msz, pr, pi)

                for m in range(NB):
                    m0, msz = BL[m]
                    wr = wslices(xr, m0, msz)
                    wi = wslices(xi, m0, msz)
                    tail = None
                    if packed_in and packing:
                        # last k-block: single 128-row packed MM (deferred -
                        # its input is written by the previous stage's
                        # partition-shift, so give it pipeline slack)
                        w_pr = wr[:-1] + wi[:-1]
                        r_pr = [
                            gr[:128, k * Wc : (k + 1) * Wc] for k in range(NB - 1)
                        ] + [
                            gni[:128, k * Wc : (k + 1) * Wc] for k in range(NB - 1)
                        ]
                        w_pi = wr[:-1] + wi[:-1]
                        r_pi = [
                            gi[:128, k * Wc : (k + 1) * Wc] for k in range(NB - 1)
                        ] + [
                            gr[:128, k * Wc : (k + 1) * Wc] for k in range(NB - 1)
                        ]
                        tail = (
                            xr[: 2 * RTL, (NB - 1) * Wc + m0 : (NB - 1) * Wc + m0 + msz],
                            pk_pr,
                            pk_pi,
                        )
                    else:
                        w_pr = wr + wi
                        r_pr = [
                            gr[:ksz, k * Wc : (k + 1) * Wc]
                            for k, (k0, ksz) in enumerate(BL)
                        ] + [
                            gni[:ksz, k * Wc : (k + 1) * Wc]
                            for k, (k0, ksz) in enumerate(BL)
                        ]
                        w_pi = wr + wi
                        r_pi = [
                            gi[:ksz, k * Wc : (k + 1) * Wc]
                            for k, (k0, ksz) in enumerate(BL)
                        ] + [
                            gr[:ksz, k * Wc : (k + 1) * Wc]
                            for k, (k0, ksz) in enumerate(BL)
                        ]

                    pr = psp.tile([128, Wc], f32, tag="mm")
                    if packed_out and packing and m == NB - 1:
                        # pack the m-tail pair into ONE psum tile: pr rows on
                        # partitions 0:msz, pi rows on 64:64+msz (the PE
                        # writes the upper column group directly - no
                        # partition-shift DMA needed at evacuation)
                        pi = None
                        out_a = pr[:msz, :]
                        out_b = pr[64 : 64 + msz, :]
                    else:
                        pi = psp.tile([128, Wc], f32, tag="mm")
                        out_a = pr[:msz, :]
                        out_b = pi[:msz, :]
                    mm_group2_head(
                        out_a, w_pr, r_pr, out_b, w_pi, r_pi,
                        has_tail=tail is not None,
                    )
                    pending.append((m, msz, pr, pi, tail))
                    import os as _os
                    if len(pending) > (0 if _os.environ.get("NO_DEFER") else 1):
                        flush_one()
                while pending:
                    flush_one()

            def evac_copy(dst_r, dst_i, packed_out, sidx=0):
                def f(m, msz, pr, pi):
                    c0 = m * Wc
                    if pi is None:
                        # packed m-tail pair in one psum tile: both halves
                        # evacuate with partition-aligned engine copies
                        a.copy(out=dst_r[:msz, c0 : c0 + Wc], in_=pr[:msz, :])
                        a.copy(
                            out=dst_r[64 : 64 + msz, c0 : c0 + Wc],
                            in_=pr[64 : 64 + msz, :],
                        )
                        return
                    a.copy(out=dst_r[:msz, c0 : c0 + Wc], in_=pr[:msz, :])
                    a.copy(out=dst_i[:msz, c0 : c0 + Wc], in_=pi[:msz, :])
                return f

            def evac_mask(dst_r, dst_i, sidx=1):
                # must be DVE: only ACT/DVE can access PSUM, and ACT cannot
                # do tensor*tensor; pair-interleaving hides the latency
                def f(m, msz, pr, pi):
                    c0 = m * Wc
                    if pi is None:
                        v.tensor_tensor(
                            out=dst_r[:msz, c0 : c0 + Wc], in0=pr[:msz, :],
                            in1=mpk[:msz, :], op=OP.mult,
                        )
                        v.tensor_tensor(
                            out=dst_r[64 : 64 + msz, c0 : c0 + Wc],
                            in0=pr[64 : 64 + msz, :],
                            in1=mpk[64 : 64 + msz, :], op=OP.mult,
                        )
                        return
                    mk = maskf[:msz, c0 : c0 + Wc]
                    v.tensor_tensor(
                        out=dst_r[:msz, c0 : c0 + Wc], in0=pr[:msz, :], in1=mk,
                        op=OP.mult,
                    )
                    v.tensor_tensor(
                        out=dst_i[:msz, c0 : c0 + Wc], in0=pi[:msz, :], in1=mk,
                        op=OP.mult,
                    )
                return f

            def proj(c):
                """Coil projection cp = p * csm (all-bf16).

                DVE carries 4 ops (2x_1p), Pool 2. Emitted ahead of the
                consuming stages so the elementwise engines compute it while
                PE runs earlier coils' stages.
                """
                slot = c % 2
                csr, csi = cs[c]
                # pj1 on Pool runs concurrently with DVE's pj2/pj3/pj4 -
                # the combine chain depth is ~2.4us instead of Pool-paced
                # 4.3us (proj gates stage 1 of every coil)
                g.tensor_tensor(out=pj1, in0=pb_r, in1=csr, op=OP.mult)
                v.tensor_tensor(out=pj2, in0=pb_i, in1=csi, op=OP.mult)
                v.tensor_tensor(out=pj3, in0=pb_r, in1=csi, op=OP.mult)
                v.tensor_tensor(out=pj4, in0=pb_i, in1=csr, op=OP.mult)
                v.tensor_tensor(out=cp[slot][0], in0=pj1, in1=pj2, op=OP.subtract)
                v.tensor_tensor(out=cp[slot][1], in0=pj3, in1=pj4, op=OP.add)
                if packing and dma_pack:
                    # pack cp: copy im tail rows into cp_r block NB-1 p64:128
                    # (partition-shifting SBUF->SBUF DMA)
                    nc.sync.dma_start(
                        out=cp[slot][0][64 : 64 + RTL, (NB - 1) * Wc : NB * Wc],
                        in_=cp[slot][1][0:RTL, (NB - 1) * Wc : NB * Wc],
                    )

            def stage(stg, c):
                slot = c % 2
                stage1_packed_in = bool(packing and dma_pack)
                if stg == 0:
                    mm_stage(cp[slot][0], cp[slot][1], gF, stage1_packed_in,
                             True, evac_copy(s1[slot][0], s1[slot][1], True, 0))
                elif stg == 1:
                    mm_stage(s1[slot][0], s1[slot][1], gF, True, True,
                             evac_mask(km[slot][0], km[slot][1]))
                elif stg == 2:
                    mm_stage(km[slot][0], km[slot][1], gB, True, True,
                             evac_copy(s3[slot][0], s3[slot][1], True, 2))
                else:
                    mm_stage(s3[slot][0], s3[slot][1], gB, True, False,
                             evac_copy(zz[slot][0], zz[slot][1], False))

            def qaccum(c):
                # q += z * conj(csm); bf16 products/pair-sums on DVE, the
                # fp32 accumulate split DVE (re) / Pool (im)
                slot = c % 2
                csr, csi = cs[c]
                zr, zi = zz[slot]
                v.tensor_tensor(out=tA, in0=zr, in1=csr, op=OP.mult)
                v.tensor_tensor(out=tB, in0=zi, in1=csi, op=OP.mult)
                v.tensor_tensor(out=tC, in0=tA, in1=tB, op=OP.add)
                v.tensor_tensor(out=q_r, in0=q_r, in1=tC, op=OP.add)
                v.tensor_tensor(out=tD, in0=zi, in1=csr, op=OP.mult)
                v.tensor_tensor(out=tE, in0=zr, in1=csi, op=OP.mult)
                v.tensor_tensor(out=tF, in0=tD, in1=tE, op=OP.subtract)
                if c == Cc - 1:
                    # the last coil's q_i accumulate gates the iteration-end
                    # reductions - keep it off the slow Pool engine
                    v.tensor_tensor(out=q_i, in0=q_i, in1=tF, op=OP.add)
                else:
                    g.tensor_tensor(out=q_i, in0=q_i, in1=tF, op=OP.add)

            # last-iteration |masked kspace|^2: ACT Square+accum, one
            # partials column per (coil, component)
            def ksq_accum(c):
                slot = c % 2
                kr, ki = km[slot]
                col = 6 + 2 * c
                a.activation(out=tA, in_=kr, func=ACTF.Square,
                             accum_out=partials[:, col : col + 1])
                a.activation(out=tB, in_=ki, func=ACTF.Square,
                             accum_out=partials[:, col + 1 : col + 2])

            for rep in range(reps):
                # ---- (re)init: r = us + mu*rec; p = r; b = 0
                if rep > 0:
                    load_folded(pp[1][0], us[0])
                    load_folded(pp[1][1], us[1])
                    load_folded(b_r, rec[0])
                    load_folded(b_i, rec[1])
                STT_V(out=r_r, in0=b_r, scalar=mu_b[:, :1], in1=pp[1][0],
                      op0=OP.mult, op1=OP.add)
                STT_V(out=r_i, in0=b_i, scalar=mu_b[:, :1], in1=pp[1][1],
                      op0=OP.mult, op1=OP.add)
                a.copy(out=pp[0][0], in_=r_r)
                a.copy(out=pp[0][1], in_=r_i)
                a.copy(out=pb_r, in_=r_r)
                a.copy(out=pb_i, in_=r_i)
                v.memset(b_r, 0.0)
                v.memset(b_i, 0.0)

                for it in range(iters):
                    p_r, p_i = pp[it % 2]
                    pn_r, pn_i = pp[(it + 1) % 2]
                    last = it + 1 == iters and rep + 1 == reps
                    # q = mu * p (coils accumulate on top); ACT scale-copy
                    if not last:
                        a.activation(out=q_r, in_=p_r, func=ACTF.Copy,
                                     scale=mu_b[:, :1])
                        a.activation(out=q_i, in_=p_i, func=ACTF.Copy,
                                     scale=mu_b[:, :1])
                    proj(0)
                    if Cc > 1:
                        proj(1)
                    n_stg = 2 if last else 4
                    # pairwise stage interleave: s1(c) s1(c+1) s2(c) s2(c+1)
                    # ... so every inter-stage evacuation hides behind the
                    # sibling coil's matmuls.
                    # qaccum emission is DEFERRED to after the NEXT pair's
                    # stage-1 mask evacuations: DVE executes FIFO, and the
                    # 7us of per-pair qaccum work (only needed at iteration
                    # end) would otherwise queue ahead of the mask evacs
                    # that gate the next pair's stage-3 matmuls.
                    pending_q = []
                    for c in range(0, Cc, 2):
                        cset = [c] + ([c + 1] if c + 1 < Cc else [])
                        for stg in range(n_stg):
                            for cc in cset:
                                stage(stg, cc)
                                if last and stg == 1:
                                    ksq_accum(cc)
                            if stg == 0:
                                # next pair's projections (cp slots freed by
                                # this pair's stage-1 reads)
                                for cc in cset:
                                    if cc + 2 < Cc:
                                        proj(cc + 2)
                            if stg == 1 and pending_q:
                                for qc in pending_q:
                                    qaccum(qc)
                                pending_q = []
                        if not last:
                            pending_q.extend(cset)
                    for qc in pending_q:
                        qaccum(qc)
                    if last:
                        # pq = sum_c ||M F C_c p||^2 + mu ||p||^2
                        pn0 = 6 + 2 * Cc
                        a.activation(out=dump, in_=p_r, func=ACTF.Square,
                                     accum_out=partials[:, pn0 : pn0 + 1])
                        a.activation(out=dump, in_=p_i, func=ACTF.Square,
                                     accum_out=partials[:, pn0 + 1 : pn0 + 2])
                        k = pn0 + 2
                        ps1 = psp.tile([1, NPART], f32, tag="mm")
                        nc.tensor.matmul(ps1[:1, :k], lhsT=ones_col[:, :1],
                                         rhs=partials[:, :k], start=True,
                                         stop=True)
                        a.copy(out=redsums[:1, :k], in_=ps1[:1, :k])
                        # sum the 2C per-coil kspace columns in one reduce
                        v.reduce_sum(out=scl[:1, 0:1],
                                     in_=redsums[:1, 6 : 6 + 2 * Cc],
                                     axis=mybir.AxisListType.X)
                        TT(out=scl[:1, 1:2], in0=redsums[:1, pn0 : pn0 + 1],
                           in1=redsums[:1, pn0 + 1 : pn0 + 2], op=OP.add)
                        # pq = ksq + mu * pnorm
                        STT_V(out=scl[:1, 2:3], in0=scl[:1, 1:2],
                              scalar=mu_sb[:1, :1], in1=scl[:1, 0:1],
                              op0=OP.mult, op1=OP.add)
                        v.reciprocal(out=scl[:1, 3:4], in_=scl[:1, 2:3])
                        TT(out=alphas[:1, 0:1], in0=rr_t[:1, :1],
                           in1=scl[:1, 3:4], op=OP.mult)      # alpha = rr/pq
                        psb2 = psp.tile([128, 16], f32, tag="mm")
                        nc.tensor.matmul(psb2[:, :1], lhsT=ones_row[:1, :128],
                                         rhs=alphas[:1, :1], start=True,
                                         stop=True)
                        a.copy(out=bc[:, :1], in_=psb2[:, :1])
                        a_ = bc[:, 0:1]
                        STT_V(out=b_r, in0=p_r, scalar=a_, in1=b_r,
                              op0=OP.mult, op1=OP.add)
                        STT_V(out=b_i, in0=p_i, scalar=a_, in1=b_i,
                              op0=OP.mult, op1=OP.add)
                        continue

                    # ---- per-sample scalars: pq (DVE mult+reduce), qq (ACT
                    # square+accum), and rr on iter 0. ACT squares emitted
                    # first so they run concurrently with the DVE pq chain.
                    a.activation(out=tA, in_=q_r, func=ACTF.Square,
                                 accum_out=partials[:, 2:3])
                    a.activation(out=tB, in_=q_i, func=ACTF.Square,
                                 accum_out=partials[:, 3:4])
                    # q_i-side product on Pool: runs concurrently with the
                    # DVE q_r-side chain at the iteration boundary
                    g.tensor_tensor(out=tC, in0=q_i, in1=p_i, op=OP.mult)
                    TT(out=dump, in0=q_r, in1=p_r, op=OP.mult)
                    v.reduce_sum(out=partials[:, 0:1], in_=dump,
                                 axis=mybir.AxisListType.X)
                    v.reduce_sum(out=partials[:, 1:2], in_=tC,
                                 axis=mybir.AxisListType.X)
                    k = 4
                    if it == 0:
                        a.activation(out=tD, in_=r_r, func=ACTF.Square,
                                     accum_out=partials[:, 4:5])
                        a.activation(out=tE, in_=r_i, func=ACTF.Square,
                                     accum_out=partials[:, 5:6])
                        k = 6
                    ps1 = psp.tile([1, 16], f32, tag="mm")
                    nc.tensor.matmul(ps1[:1, :k], lhsT=ones_col[:, :1],
                                     rhs=partials[:, :k], start=True, stop=True)
                    a.copy(out=redsums[:1, :k], in_=ps1[:1, :k])
                    # pq = c0+c1, qq = c2+c3 (, rr = c4+c5)
                    TT(out=scl[:1, 0:1], in0=redsums[:1, 0:1],
                       in1=redsums[:1, 1:2], op=OP.add)
                    TT(out=scl[:1, 1:2], in0=redsums[:1, 2:3],
                       in1=redsums[:1, 3:4], op=OP.add)
                    if it == 0:
                        TT(out=rr_t[:1, :1], in0=redsums[:1, 4:5],
                           in1=redsums[:1, 5:6], op=OP.add)
                    v.reciprocal(out=scl[:1, 2:3], in_=scl[:1, 0:1])
                    TT(out=alphas[:1, 0:1], in0=rr_t[:1, :1],
                       in1=scl[:1, 2:3], op=OP.mult)          # alpha = rr/pq
                    TT(out=scl[:1, 3:4], in0=alphas[:1, 0:1],
                       in1=alphas[:1, 0:1], op=OP.mult)       # alpha^2
                    TT(out=scl[:1, 4:5], in0=scl[:1, 3:4],
                       in1=scl[:1, 1:2], op=OP.mult)          # alpha^2 qq
                    TT(out=rrn_t[:1, :1], in0=scl[:1, 4:5],
                       in1=rr_t[:1, :1], op=OP.subtract)      # rr_new
                    v.reciprocal(out=scl[:1, 5:6], in_=rr_t[:1, :1])
                    TT(out=alphas[:1, 2:3], in0=rrn_t[:1, :1],
                       in1=scl[:1, 5:6], op=OP.mult)          # beta
                    v.tensor_scalar_mul(out=alphas[:1, 1:2],
                                        in0=alphas[:1, 0:1], scalar1=-1.0)
                    a.copy(out=rr_t[:1, :1], in_=rrn_t[:1, :1])
                    psb2 = psp.tile([128, 16], f32, tag="mm")
                    nc.tensor.matmul(psb2[:, :3], lhsT=ones_row[:1, :128],
                                     rhs=alphas[:1, :3], start=True, stop=True)
                    a.copy(out=bc[:, :3], in_=psb2[:, :3])
                    a_ = bc[:, 0:1]
                    na = bc[:, 1:2]
                    bet = bc[:, 2:3]
                    # r -= alpha q ; p' = r + beta p ; b += alpha p (reads old
                    # p, emitted last - it doesn't gate the next iteration).
                    STT_V(out=r_r, in0=q_r, scalar=na, in1=r_r,
                          op0=OP.mult, op1=OP.add)
                    STT_V(out=pn_r, in0=p_r, scalar=bet, in1=r_r,
                          op0=OP.mult, op1=OP.add)
                    a.copy(out=pb_r, in_=pn_r)
                    STT_V(out=r_i, in0=q_i, scalar=na, in1=r_i,
                          op0=OP.mult, op1=OP.add)
                    STT_V(out=pn_i, in0=p_i, scalar=bet, in1=r_i,
                          op0=OP.mult, op1=OP.add)
                    a.copy(out=pb_i, in_=pn_i)
                    STT_V(out=b_r, in0=p_r, scalar=a_, in1=b_r,
                          op0=OP.mult, op1=OP.add)
                    STT_V(out=b_i, in0=p_i, scalar=a_, in1=b_i,
                          op0=OP.mult, op1=OP.add)

            store_folded(b_r, out_d[0])
            store_folded(b_i, out_d[1])

    nc.compile()
    return nc


def _host_inputs(Hc, Wc, packing, RTL, NB):
    bf = np.float16
    Fc = _centered_dft(Hc)
    f_r = np.ascontiguousarray(Fc.real).astype(np.float32)
    f_i = np.ascontiguousarray(Fc.imag).astype(np.float32)
    f_ni = (-f_i).astype(np.float32)
    shared = {"f_r": f_r.astype(bf), "f_i": f_i.astype(bf), "f_ni": f_ni.astype(bf)}
    if packing:
        t0 = 128 * (NB - 1)
        fr2 = f_r[t0:Hc, :]
        fi2 = f_i[t0:Hc, :]
        fni2 = f_ni[t0:Hc, :]
        fpk = np.concatenate(
            [
                np.concatenate([fr2, fni2], axis=0),
                np.concatenate([fi2, fr2], axis=0),
                np.concatenate([fr2, fi2], axis=0),
                np.concatenate([fni2, fr2], axis=0),
            ],
            axis=1,
        )
        shared["f_pk"] = np.ascontiguousarray(fpk).astype(bf)
    return shared


def prepare(us_image, reconstruction, mask, csm_r, csm_i, mu, reps=1):
    """Build (cached) the Bass module and per-core input maps."""
    bf = np.float16
    Bc, _, Hc, Wc = us_image.shape
    Cc = csm_r.shape[1]
    n_cores = Bc
    iters = CG_ITER

    BL = _blocks(Hc)
    NB = len(BL)
    RTL = BL[-1][1] if BL[-1][1] < 128 else 0
    packing = RTL > 0 and 2 * RTL <= 128

    key = (Hc, Wc, Cc, iters, n_cores, reps)
    if key not in _nc_cache:
        _nc_cache[key] = _build(Hc, Wc, Cc, iters, n_cores, reps=reps)
    nc = _nc_cache[key]

    shared = _host_inputs(Hc, Wc, packing, RTL, NB)

    in_maps = []
    for b in range(n_cores):
        m = {
            "us_image": np.ascontiguousarray(us_image[b], dtype=np.float32),
            "reconstruction": np.ascontiguousarray(
                reconstruction[b], dtype=np.float32
            ),
            "mask": np.ascontiguousarray(mask[b, 0]).astype(bf),
            "csm_r": np.ascontiguousarray(csm_r[b]).astype(bf),
            "csm_i": np.ascontiguousarray(csm_i[b]).astype(bf),
            "mu": np.ascontiguousarray(mu, dtype=np.float32),
        }
        m.update(shared)
        if packing:
            t0 = 128 * (NB - 1)
            m2 = np.ascontiguousarray(mask[b, 0, t0:Hc, :])
            m["mask_pk"] = np.concatenate([m2, m2], axis=0).astype(bf)
        in_maps.append(m)
    return in_maps, nc, n_cores


def kernel(us_image, reconstruction, mask, csm_r, csm_i, mu):
    global LAST_RESULT
    from concourse.bass_utils import run_bass_kernel_spmd

    in_maps, nc, n_cores = prepare(us_image, reconstruction, mask, csm_r, csm_i, mu)
    res = run_bass_kernel_spmd(nc, in_maps, core_ids=list(range(n_cores)))
    LAST_RESULT = res
    out = np.stack([res.results[b]["out"] for b in range(n_cores)], axis=0)
    return out.astype(np.float32)
